# revision 1
# baseline (speedup 1.0000x reference)
"""Trainium2 Bass kernel for nn_CrossFramelAttentionBlock.

Data-parallel over the bt=32 batch-columns: 8 cores x 4 columns each, no
collectives. The tiny msg (CLS cross-frame) attention mixes only the T=8
frames of one batch element b; each core computes it for its own b (8 CLS
tokens fed as a per-core input, rotated so this core's 4 columns are rows
0..3 -- attention is permutation-equivariant, so the rotation is sound).

Layout strategy: activations token-major (LN/softmax use per-partition
scalars), feature-major PE-transposed copies feed matmuls. Big GEMMs bf16
with fp32 PSUM accumulation; softmax without max-subtraction (activations
are O(1)); attention denominators via a ones-column appended to V; MoE
routing applied input-side (head) / output-side (tail); biases on
token-major outputs via K=1 matmuls with bias rows.
"""

import numpy as np
import ml_dtypes

import concourse.bass as bass
import concourse.tile as tile
from concourse import mybir, bacc
from concourse.bass_utils import run_bass_kernel_spmd

F32 = mybir.dt.float32
BF16 = mybir.dt.bfloat16
AF = mybir.ActivationFunctionType
OP = mybir.AluOpType

D = 768
H = 12
T = 8
E1 = 5          # E + 1 experts (shared c_fc/c_proj is expert 0)
F = 3072
L = 197
LC = 198        # L + msg token
BT = 32
NCORES = 8
CPC = 4         # columns per core
DC = 6          # 128-chunks of D
FC = 24         # 128-chunks of F
NT = CPC * LC   # 792 attention tokens per core
NT2 = CPC * L   # 788 moe tokens per core
MH = NT2 // 2   # 394

ACHUNKS = [(0, 128), (128, 70)]   # attention token chunks (incl. msg token)
MCHUNKS = [(0, 128), (128, 69)]   # moe / output chunks (msg token excluded)

_CACHE = {}


def _bcast_row_ap(dram_ap, n):
    """DRAM AP of a 1-D tensor broadcast across n partitions."""
    return bass.AP(tensor=dram_ap.tensor, offset=dram_ap.offset,
                   ap=[[0, n]] + list(dram_ap.ap))


def build(reps=1):
    nc = bacc.Bacc()

    def inp(name, shape, dt=BF16):
        return nc.declare_dram_parameter(name, list(shape), dt, isOutput=False)

    xs = inp("xs", (L, CPC, D), F32)
    x0c = inp("x0c", (T, D), F32)       # this core's b CLS rows, rotated
    id32 = inp("id32", (128, 128), F32)
    id16 = inp("id16", (128, 128), BF16)
    mfcwT = inp("mfcwT", (D, D))
    mqkvT = inp("mqkvT", (D, 3 * D))
    mwoT = inp("mwoT", (D, D))
    wqkvT = inp("wqkvT", (D, 3 * D))
    woT = inp("woT", (D, D))
    whT = inp("whT", (E1, D, F))
    wtT = inp("wtT", (E1, F, D))
    r1wT = inp("r1wT", (D, E1))
    r2wT = inp("r2wT", (F, E1))
    bhE = inp("bhE", (E1, F))
    btE = inp("btE", (E1, D))
    mfcb_r = inp("mfcb_r", (1, D))
    mqkvb_r = inp("mqkvb_r", (1, 3 * D))
    mob_r = inp("mob_r", (1, D))
    wob_r = inp("wob_r", (1, D))
    r1b_r = inp("r1b_r", (1, E1))
    r2b_r = inp("r2b_r", (1, E1))
    qkvb = inp("qkvb", (3 * D,), F32)
    mlng = inp("mlng", (D,), F32)
    mlnb = inp("mlnb", (D,), F32)
    ln1g = inp("ln1g", (D,), F32)
    ln1b = inp("ln1b", (D,), F32)
    ln2g = inp("ln2g", (D,), F32)
    ln2b = inp("ln2b", (D,), F32)

    out = nc.declare_dram_parameter("out", [L, CPC, D], F32, isOutput=True)

    with tile.TileContext(nc) as tc:
        def _body(rep):
            def pool(name, bufs, space="SBUF", side=None):
                kw = {"side": side} if side else {}
                return tc.alloc_tile_pool(name=f"{name}_{rep}", bufs=bufs,
                                          space=space, **kw)

            persist = pool("persist", 1)
            consts = pool("consts", 1)
            lnscr = pool("lnscr", 2)

            # ---------------- global constants ----------------
            id32_t = consts.tile([128, 128], F32, name="id32_t")
            nc.sync.dma_start(out=id32_t[:], in_=id32[:])
            id16_t = consts.tile([128, 128], BF16, name="id16_t")
            nc.sync.dma_start(out=id16_t[:], in_=id16[:])
            ones_r = consts.tile([1, 128], BF16, name="ones_r")
            nc.vector.memset(ones_r, 1.0)
            eps_t = consts.tile([128, 1], F32, name="eps_t")
            nc.vector.memset(eps_t, 1e-5)
            qkvb_t = consts.tile([128, 18], F32, name="qkvb_t")
            nc.sync.dma_start(out=qkvb_t[:], in_=qkvb[:].rearrange("(c p) -> p c", p=128))

            def brow_tile(p, dram, n, nm):
                rt = p.tile([1, n], BF16, tag=f"brow_{nm}", name=f"brow_{nm}")
                nc.sync.dma_start(out=rt[:], in_=dram[:])
                return rt

            wob_t = brow_tile(consts, wob_r, D, "wob")
            r1b_t = brow_tile(consts, r1b_r, E1, "r1b")
            r2b_t = brow_tile(consts, r2b_r, E1, "r2b")

            def ln_params(p, g_d, b_d):
                gt = p.tile([128, D], F32, tag=f"lnp_{g_d.name}", name=f"lnp_{g_d.name}")
                nc.gpsimd.dma_start(out=gt[:], in_=_bcast_row_ap(g_d[:], 128))
                bt_ = p.tile([128, D], F32, tag=f"lnp_{b_d.name}", name=f"lnp_{b_d.name}")
                nc.gpsimd.dma_start(out=bt_[:], in_=_bcast_row_ap(b_d[:], 128))
                return gt, bt_

            # ---------------- load x (token-major, fp32) ----------------
            xc_tok = []  # [c][lc] -> [128, 768] fp32 tile (lc1: rows 0:70, row 69 = msg)
            for c in range(CPC):
                col = []
                for (l0, nl) in ACHUNKS:
                    t_ = persist.tile([128, D], F32, tag=f"xc_{c}_{l0}", name=f"xc_{c}_{l0}")
                    nld = min(l0 + nl, L) - l0
                    nc.sync.dma_start(out=t_[0:nld, :], in_=xs[l0:l0 + nld, c, :])
                    col.append(t_)
                xc_tok.append(col)

            # ---------------- helpers ----------------
            def layernorm(src_ap, nl, g_t, b_t, dst_ap):
                """token-major LN: src [nl, 768] fp32 -> dst [nl, 768] bf16"""
                stats = lnscr.tile([128, 3, 6], F32, tag="ln_stats", name="ln_stats")
                sr = src_ap.rearrange("p (c f) -> p c f", c=3)
                for cc in range(3):
                    nc.vector.bn_stats(out=stats[0:nl, cc, :], in_=sr[:, cc, :])
                mv = lnscr.tile([128, 2], F32, tag="ln_mv", name="ln_mv")
                nc.vector.bn_aggr(out=mv[0:nl, :], in_=stats[0:nl, :, :])
                rstd = lnscr.tile([128, 1], F32, tag="ln_rstd", name="ln_rstd")
                nc.scalar.activation(out=rstd[0:nl, :], in_=mv[0:nl, 1:2],
                                     func=AF.Sqrt, bias=eps_t[0:nl, :], scale=1.0)
                nc.vector.reciprocal(out=rstd[0:nl, :], in_=rstd[0:nl, :])
                tmp = lnscr.tile([128, D], F32, tag="ln_tmp", name="ln_tmp")
                nc.vector.tensor_scalar(out=tmp[0:nl, :], in0=src_ap,
                                        scalar1=mv[0:nl, 0:1], scalar2=rstd[0:nl, :],
                                        op0=OP.subtract, op1=OP.mult)
                nc.vector.tensor_mul(out=tmp[0:nl, :], in0=tmp[0:nl, :], in1=g_t[0:nl, :])
                nc.vector.tensor_add(out=dst_ap, in0=tmp[0:nl, :], in1=b_t[0:nl, :])

            def tp16(psum_pool, src_ap, np_, nf, dst_ap, tagp="tp"):
                """bf16 transpose: src [np_, nf] -> dst [nf, np_] (PE + copy)"""
                ps = psum_pool.tile([128, 128], BF16, tag=tagp, name=tagp)
                nc.tensor.transpose(ps[0:nf, 0:np_], src_ap, id16_t[0:np_, 0:np_])
                nc.any.tensor_copy(out=dst_ap, in_=ps[0:nf, 0:np_])


            # =========================================================
            # msg path: one b, 8 CLS tokens (rows 0..3 = this core's columns)
            # =========================================================
            msgp = pool("msgp", 1)
            msgh = pool("msgh", 4)
            msps = pool("msps", 4, "PSUM")
            mlng_t, mlnb_t = ln_params(msgp, mlng, mlnb)
            mfcb_t = brow_tile(msgp, mfcb_r, D, "mfcb")
            mqkvb_t = brow_tile(msgp, mqkvb_r, 3 * D, "mqkvb")
            mob_t = brow_tile(msgp, mob_r, D, "mob")

            x0_t = msgp.tile([T, D], F32, name="x0_t")
            nc.sync.dma_start(out=x0_t[:], in_=x0c[:])
            x0_b = msgp.tile([T, D], BF16, name="x0_b")
            nc.vector.tensor_copy(out=x0_b[:], in_=x0_t[:])
            x0T = msgp.tile([128, DC, T], BF16, name="x0T")
            for kc in range(DC):
                tp16(msps, x0_b[:, kc * 128:(kc + 1) * 128], T, 128, x0T[:, kc, :], "msg_ps")

            mfcw_t = msgp.tile([128, DC, D], BF16, name="mfcw_t")
            nc.sync.dma_start(out=mfcw_t[:], in_=mfcwT[:].rearrange("(kc p) o -> p kc o", p=128))

            m0 = msgp.tile([T, D], F32, name="m0")
            for oc in range(2):
                osl = slice(oc * 384, (oc + 1) * 384)
                ps = msps.tile([T, 384], F32, tag="msg_ps", name="msg_ps")
                for kc in range(DC):
                    nc.tensor.matmul(ps[:], x0T[:, kc, :], mfcw_t[:, kc, osl],
                                     start=(kc == 0), stop=False)
                nc.tensor.matmul(ps[:], ones_r[0:1, 0:T], mfcb_t[0:1, osl],
                                 start=False, stop=True)
                nc.vector.tensor_copy(out=m0[:, osl], in_=ps[:])

            mln = msgp.tile([T, D], BF16, name="mln")
            layernorm(m0[:], T, mlng_t, mlnb_t, mln[0:T, :])
            mlnT = msgp.tile([128, DC, T], BF16, name="mlnT")
            for kc in range(DC):
                tp16(msps, mln[:, kc * 128:(kc + 1) * 128], T, 128, mlnT[:, kc, :], "msg_ps")

            mqkv_t = msgp.tile([128, DC, 3 * D], BF16, name="mqkv_t")
            nc.sync.dma_start(out=mqkv_t[:], in_=mqkvT[:].rearrange("(kc p) o -> p kc o", p=128))
            qkv_m = msgp.tile([T, 3 * D], BF16, name="qkv_m")
            for oc in range(6):
                osl = slice(oc * 384, (oc + 1) * 384)
                ps = msps.tile([T, 384], F32, tag="msg_ps", name="msg_ps")
                for kc in range(DC):
                    nc.tensor.matmul(ps[:], mlnT[:, kc, :], mqkv_t[:, kc, osl],
                                     start=(kc == 0), stop=False)
                nc.tensor.matmul(ps[:], ones_r[0:1, 0:T], mqkvb_t[0:1, osl],
                                 start=False, stop=True)
                nc.vector.tensor_copy(out=qkv_m[:, osl], in_=ps[:])

            mo = msgp.tile([T, D], BF16, name="mo")
            for h in range(H):
                q_sl = qkv_m[:, h * 64:(h + 1) * 64]
                k_sl = qkv_m[:, D + h * 64: D + (h + 1) * 64]
                v_sl = qkv_m[:, 2 * D + h * 64: 2 * D + (h + 1) * 64]
                qT = msgh.tile([64, T], BF16, tag="ms_qT", name="ms_qT")
                tp16(msps, q_sl, T, 64, qT[:], "msg_ps")
                kT = msgh.tile([64, T], BF16, tag="ms_kT", name="ms_kT")
                tp16(msps, k_sl, T, 64, kT[:], "msg_ps")
                ps_s = msps.tile([T, T], F32, tag="msg_ps", name="msg_ps")
                nc.tensor.matmul(ps_s[:], qT[:], kT[:], start=True, stop=True)
                e_t = msgh.tile([T, T], BF16, tag="ms_e", name="ms_e")
                den = msgh.tile([T, 1], F32, tag="ms_den", name="ms_den")
                nc.scalar.activation(out=e_t[:], in_=ps_s[:], func=AF.Exp,
                                     scale=0.125, accum_out=den[:])
                rd = msgh.tile([T, 1], F32, tag="ms_rd", name="ms_rd")
                nc.vector.reciprocal(out=rd[:], in_=den[:])
                p_t = msgh.tile([T, T], BF16, tag="ms_p", name="ms_p")
                nc.vector.tensor_scalar_mul(out=p_t[:], in0=e_t[:], scalar1=rd[:])
                pT = msgh.tile([T, T], BF16, tag="ms_pT", name="ms_pT")
                tp16(msps, p_t[:], T, T, pT[:], "msg_ps")
                ps_o = msps.tile([T, 64], F32, tag="msg_ps", name="msg_ps")
                nc.tensor.matmul(ps_o[:], pT[:], v_sl, start=True, stop=True)
                nc.any.tensor_copy(out=mo[:, h * 64:(h + 1) * 64], in_=ps_o[:])

            moT = msgp.tile([128, DC, T], BF16, name="moT")
            for kc in range(DC):
                tp16(msps, mo[:, kc * 128:(kc + 1) * 128], T, 128, moT[:, kc, :], "msg_ps")
            mwo_t = msgp.tile([128, DC, D], BF16, name="mwo_t")
            nc.sync.dma_start(out=mwo_t[:], in_=mwoT[:].rearrange("(kc p) o -> p kc o", p=128))
            msg_tok = persist.tile([T, D], F32, name="msg_tok")
            for oc in range(2):
                osl = slice(oc * 384, (oc + 1) * 384)
                ps = msps.tile([T, 384], F32, tag="msg_ps", name="msg_ps")
                for kc in range(DC):
                    nc.tensor.matmul(ps[:], moT[:, kc, :], mwo_t[:, kc, osl],
                                     start=(kc == 0), stop=False)
                nc.tensor.matmul(ps[:], ones_r[0:1, 0:T], mob_t[0:1, osl],
                                 start=False, stop=True)
                nc.vector.tensor_add(out=msg_tok[:, osl], in0=m0[:, osl], in1=ps[:])

            for c in range(CPC):
                nc.sync.dma_start(out=xc_tok[c][1][69:70, :], in_=msg_tok[c:c + 1, :])
            msps.release()
            msgh.release()
            msgp.release()
            # =========================================================
            # LN1 (regular tokens) -> ln1T [kc][128, 792] bf16
            # free layout: [0:788] regular tokens (c*197+l), [788:792] msg
            # =========================================================
            ln1T_pool = pool("ln1Tp", 1)
            cln1 = pool("cln1", 1)
            ln1g_t, ln1b_t = ln_params(cln1, ln1g, ln1b)
            ps_ln1 = pool("ps_ln1", 3, "PSUM")

            ln1T = [ln1T_pool.tile([128, NT], BF16, tag=f"ln1T_{kc}", name=f"ln1T_{kc}")
                    for kc in range(DC)]
            for c in range(CPC):
                for lc, (l0, nl) in enumerate(MCHUNKS):
                    lnb = lnscr.tile([128, D], BF16, tag="ln_tok", name="ln_tok")
                    layernorm(xc_tok[c][lc][0:nl, :], nl, ln1g_t, ln1b_t, lnb[0:nl, :])
                    for kc in range(DC):
                        tp16(ps_ln1, lnb[0:nl, kc * 128:(kc + 1) * 128], nl, 128,
                             ln1T[kc][:, c * L + l0: c * L + l0 + nl], "ln_tp")

            # =========================================================
            # main qkv pass A (788 regular tokens; no msg dependency)
            # qkvT keeps the interleaved (c*198+l) layout for attention
            # =========================================================
            qkvT_pool = pool("qkvTp", 1, side="right")
            wq_pool = pool("wqp", 1)
            ps_qkv = pool("ps_qkv", 4, "PSUM")

            wq_t = [wq_pool.tile([128, DC, 128], BF16, tag=f"wq_{oc}", name=f"wq_{oc}")
                    for oc in range(18)]
            for oc in range(18):
                nc.sync.dma_start(
                    out=wq_t[oc][:],
                    in_=wqkvT[:, oc * 128:(oc + 1) * 128].rearrange("(kc p) o -> p kc o", p=128))

            qkvT = [qkvT_pool.tile([128, NT], BF16, tag=f"qkvT_{oc}", name=f"qkvT_{oc}")
                    for oc in range(18)]
            for oc in range(18):
                for hf in range(2):
                    tsl = slice(hf * MH, (hf + 1) * MH)
                    ps = ps_qkv.tile([128, MH], F32, tag="qkv_ps", name="qkv_ps")
                    for kc in range(DC):
                        nc.tensor.matmul(ps[:], wq_t[oc][:, kc, :], ln1T[kc][:, tsl],
                                         start=(kc == 0), stop=(kc == DC - 1))
                    for ci in range(2):
                        c = hf * 2 + ci
                        nc.scalar.activation(
                            out=qkvT[oc][:, c * LC: c * LC + L],
                            in_=ps[:, ci * L:(ci + 1) * L],
                            func=AF.Identity, bias=qkvb_t[:, oc:oc + 1], scale=1.0)


            # ---- LN1 of the 4 msg tokens + qkv pass B ----
            lnb4 = lnscr.tile([128, D], BF16, tag="ln_tok", name="ln_tok")
            layernorm(msg_tok[0:CPC, :], CPC, ln1g_t, ln1b_t, lnb4[0:CPC, :])
            for kc in range(DC):
                tp16(ps_ln1, lnb4[0:CPC, kc * 128:(kc + 1) * 128], CPC, 128,
                     ln1T[kc][:, NT2:NT2 + CPC], "ln_tp")
            for oc in range(18):
                ps = ps_qkv.tile([128, CPC], F32, tag="qkv_ps", name="qkv_ps")
                for kc in range(DC):
                    nc.tensor.matmul(ps[:], wq_t[oc][:, kc, :], ln1T[kc][:, NT2:NT2 + CPC],
                                     start=(kc == 0), stop=(kc == DC - 1))
                nc.scalar.activation(
                    out=qkvT[oc].rearrange("p (c l) -> p c l", c=CPC)[:, :, L],
                    in_=ps[:], func=AF.Identity, bias=qkvb_t[:, oc:oc + 1], scale=1.0)
            ps_qkv.release()
            wq_pool.release()
            ps_ln1.release()
            cln1.release()
            ln1T_pool.release()

            # =========================================================
            # attention per (c, h) -> att_tok [c][lc][128, 768] bf16
            # =========================================================
            att_pool = pool("attp", 1)
            atp = pool("atp", 3)
            psA = pool("psA", 2, "PSUM")
            psB = pool("psB", 2, "PSUM")

            att_tok = [[att_pool.tile([128, D], BF16, tag=f"att_{c}_{lc}", name=f"att_{c}_{lc}")
                        for lc in range(2)] for c in range(CPC)]
            oap = pool("oap", 1)
            oa_tiles = {}
            # chunk-0 (j=0..127): depends only on qkv pass A -> fills msg bubble
            for c in range(CPC):
                csl0 = slice(c * LC, c * LC + L)   # i-range excludes msg token
                for h in range(H):
                    tq, of = h // 2, (h % 2) * 64
                    qT = qkvT[tq][of:of + 64, csl0]
                    kT = qkvT[6 + tq][of:of + 64, c * LC: (c + 1) * LC]
                    vT = qkvT[12 + tq][of:of + 64, c * LC: (c + 1) * LC]
                    e_t = atp.tile([128, L], BF16, tag="at_e", name="at_e")
                    ps_s = psA.tile([128, L], F32, tag="at_s", name="at_s")
                    nc.tensor.matmul(ps_s[:], kT[:, 0:128], qT, start=True, stop=True)
                    nc.scalar.activation(out=e_t[:], in_=ps_s[:],
                                         func=AF.Exp, scale=0.125)
                    v_t = atp.tile([128, 65], BF16, tag="at_v", name="at_v")
                    nc.vector.memset(v_t[:, 64:65], 1.0)
                    ps_v = psB.tile([128, 64], BF16, tag="at_vps", name="at_vps")
                    nc.tensor.transpose(ps_v[:, :], vT[:, 0:128],
                                        id16_t[of:of + 64, of:of + 64])
                    nc.any.tensor_copy(out=v_t[:, 0:64], in_=ps_v[:, :])
                    ps_oa = psA.tile([65, L], F32, tag="at_oa", name="at_oa")
                    nc.tensor.matmul(ps_oa[:], v_t[:], e_t[:], start=True, stop=True)
                    oa = oap.tile([65, L], F32, tag=f"oa_{c}_{h}", name=f"oa_{c}_{h}")
                    nc.vector.tensor_copy(out=oa[:], in_=ps_oa[:])
                    oa_tiles[(c, h)] = oa
            # chunk-1 (j=128..197 incl msg token): needs qkv pass B
            for c in range(CPC):
                csl0 = slice(c * LC, c * LC + L)
                for h in range(H):
                    tq, of = h // 2, (h % 2) * 64
                    qT = qkvT[tq][of:of + 64, csl0]
                    kT = qkvT[6 + tq][of:of + 64, c * LC: (c + 1) * LC]
                    vT = qkvT[12 + tq][of:of + 64, c * LC: (c + 1) * LC]
                    oa = oa_tiles[(c, h)]
                    e_t = atp.tile([128, L], BF16, tag="at_e", name="at_e")
                    ps_s = psA.tile([128, L], F32, tag="at_s", name="at_s")
                    nc.tensor.matmul(ps_s[0:70, :], kT[:, 128:LC], qT,
                                     start=True, stop=True)
                    nc.scalar.activation(out=e_t[0:70, :], in_=ps_s[0:70, :],
                                         func=AF.Exp, scale=0.125)
                    v_t = atp.tile([128, 65], BF16, tag="at_v", name="at_v")
                    nc.vector.memset(v_t[:, 64:65], 1.0)
                    ps_v = psB.tile([128, 64], BF16, tag="at_vps", name="at_vps")
                    nc.tensor.transpose(ps_v[0:70, :], vT[:, 128:LC],
                                        id16_t[of:of + 64, of:of + 64])
                    nc.any.tensor_copy(out=v_t[0:70, 0:64], in_=ps_v[0:70, :])
                    ps_oa = psA.tile([65, L], F32, tag="at_oa", name="at_oa")
                    nc.tensor.matmul(ps_oa[:], v_t[0:70, :], e_t[0:70, :],
                                     start=True, stop=True)
                    nc.vector.tensor_add(out=oa[:], in0=oa[:], in1=ps_oa[:])
                    for lc, (l0, nl) in enumerate(MCHUNKS):
                        ps_ot = psB.tile([128, 65], F32, tag="at_ot", name="at_ot")
                        nc.tensor.transpose(ps_ot[0:nl, :], oa[:, l0:l0 + nl],
                                            id32_t[0:65, 0:65])
                        rd = atp.tile([128, 1], F32, tag="at_rd", name="at_rd")
                        nc.vector.reciprocal(out=rd[0:nl, :], in_=ps_ot[0:nl, 64:65])
                        nc.vector.tensor_scalar_mul(
                            out=att_tok[c][lc][0:nl, h * 64:(h + 1) * 64],
                            in0=ps_ot[0:nl, 0:64], scalar1=rd[0:nl, :])
            oap.release()
            psB.release()
            psA.release()
            atp.release()
            qkvT_pool.release()

            # =========================================================
            # attention out-proj (token-major) + residual into xc_tok
            # =========================================================
            attT_pool = pool("attTp", 1)
            wo_pool = pool("wop", 1)
            ps_at = pool("ps_at", 4, "PSUM")
            ps_pr = pool("ps_pr", 4, "PSUM")

            attT = [attT_pool.tile([128, NT], BF16, tag=f"attT_{kc}", name=f"attT_{kc}")
                    for kc in range(DC)]
            for c in range(CPC):
                for lc, (l0, nl) in enumerate(MCHUNKS):
                    for kc in range(DC):
                        tp16(ps_at, att_tok[c][lc][0:nl, kc * 128:(kc + 1) * 128], nl, 128,
                             attT[kc][:, c * LC + l0: c * LC + l0 + nl], "ln_tp")

            wo_t = wo_pool.tile([128, DC, D], BF16, name="wo_t")
            nc.sync.dma_start(out=wo_t[:], in_=woT[:].rearrange("(kc p) o -> p kc o", p=128))
            for c in range(CPC):
                for lc, (l0, nl) in enumerate(MCHUNKS):
                    tb = c * LC + l0
                    for oc in range(2):
                        osl = slice(oc * 384, (oc + 1) * 384)
                        ps = ps_pr.tile([128, 384], F32, tag="pr_ps", name="pr_ps")
                        for kc in range(DC):
                            nc.tensor.matmul(ps[0:nl, :], attT[kc][:, tb:tb + nl],
                                             wo_t[:, kc, osl], start=(kc == 0), stop=False)
                        nc.tensor.matmul(ps[0:nl, :], ones_r[0:1, 0:nl], wob_t[0:1, osl],
                                         start=False, stop=True)
                        nc.vector.tensor_add(out=xc_tok[c][lc][0:nl, osl],
                                             in0=xc_tok[c][lc][0:nl, osl], in1=ps[0:nl, :])
            ps_pr.release()
            ps_at.release()
            wo_pool.release()
            attT_pool.release()
            att_pool.release()

            # =========================================================
            # LN2 -> ln2T [kc][128, 788] bf16  (msg tokens excluded)
            # =========================================================
            wtp = pool("wtp", 3)
            whp = pool("whp", 10)
            ln2T_pool = pool("ln2Tp", 1)
            cln2 = pool("cln2", 1)
            ln2g_t, ln2b_t = ln_params(cln2, ln2g, ln2b)
            ps_ln2 = pool("ps_ln2", 6, "PSUM")

            ln2T = [ln2T_pool.tile([128, NT2], BF16, tag=f"ln2T_{kc}", name=f"ln2T_{kc}")
                    for kc in range(DC)]
            for c in range(CPC):
                for lc, (l0, nl) in enumerate(MCHUNKS):
                    lnb = lnscr.tile([128, D], BF16, tag="ln_tok", name="ln_tok")
                    layernorm(xc_tok[c][lc][0:nl, :], nl, ln2g_t, ln2b_t, lnb[0:nl, :])
                    for kc in range(DC):
                        tp16(ps_ln2, lnb[0:nl, kc * 128:(kc + 1) * 128], nl, 128,
                             ln2T[kc][:, c * L + l0: c * L + l0 + nl], "ln_tp")
            ps_ln2.release()
            cln2.release()

            # =========================================================
            # routers + MoE
            # =========================================================
            rper = pool("rper", 1, side="right")
            ohT_pool = pool("ohTp", 1, side="right")
            r1w_t = rper.tile([128, DC, E1], BF16, name="r1w_t")
            nc.sync.dma_start(out=r1w_t[:], in_=r1wT[:].rearrange("(kc p) e -> p kc e", p=128))
            r2w_t = rper.tile([128, FC, E1], BF16, name="r2w_t")
            nc.sync.dma_start(out=r2w_t[:], in_=r2wT[:].rearrange("(kc p) e -> p kc e", p=128))
            bh_t = rper.tile([E1, F], BF16, name="bh_t")
            nc.sync.dma_start(out=bh_t[:], in_=bhE[:])
            bt_t = rper.tile([E1, D], BF16, name="bt_t")
            nc.sync.dma_start(out=bt_t[:], in_=btE[:])

            def router(ps_r, src_T, w_t, nkc, bias_row, dstT):
                """softmax(x @ rw.T + rb) token-major -> transposed into dstT [5, NT2]"""
                for c in range(CPC):
                    for lc, (l0, nl) in enumerate(MCHUNKS):
                        tb = c * L + l0
                        ps = ps_r.tile([128, E1], F32, tag="r_ps", name="r_ps")
                        for kc in range(nkc):
                            nc.tensor.matmul(ps[0:nl, :], src_T[kc][:, tb:tb + nl],
                                             w_t[:, kc, :], start=(kc == 0), stop=False)
                        nc.tensor.matmul(ps[0:nl, :], ones_r[0:1, 0:nl], bias_row[0:1, :],
                                         start=False, stop=True)
                        er = lnscr.tile([128, E1], BF16, tag="r_e", name="r_e")
                        den = lnscr.tile([128, 1], F32, tag="r_den", name="r_den")
                        nc.scalar.activation(out=er[0:nl, :], in_=ps[0:nl, :],
                                             func=AF.Exp, accum_out=den[0:nl, :])
                        rdd = lnscr.tile([128, 1], F32, tag="r_rd", name="r_rd")
                        nc.vector.reciprocal(out=rdd[0:nl, :], in_=den[0:nl, :])
                        rn = lnscr.tile([128, E1], BF16, tag="r_n", name="r_n")
                        nc.vector.tensor_scalar_mul(out=rn[0:nl, :], in0=er[0:nl, :],
                                                    scalar1=rdd[0:nl, :])
                        tp16(ps_r, rn[0:nl, :], nl, E1, dstT[0:E1, tb:tb + nl], "r_tp")

            # ---- router 1 + xeT ----
            ps_r1 = pool("ps_r1", 2, "PSUM")
            r1nT = rper.tile([E1, NT2], BF16, name="r1nT")
            router(ps_r1, ln2T, r1w_t, DC, r1b_t, r1nT)
            r1row = [rper.tile([1, NT2], BF16, tag=f"r1row_{e}", name=f"r1row_{e}")
                     for e in range(E1)]
            for e in range(E1):
                nc.sync.dma_start(out=r1row[e][0:1, :], in_=r1nT[e:e + 1, :])

            xeT_pool = pool("xeTp", 1, side="right")
            xeT = [[xeT_pool.tile([128, NT2], BF16, tag=f"xeT_{e}_{kc}", name=f"xeT_{e}_{kc}")
                    for kc in range(DC)] for e in range(E1)]
            for e in range(E1):
                for hf in range(2):
                    tsl = slice(hf * MH, (hf + 1) * MH)
                    ps_bc = ps_r1.tile([128, MH], F32, tag="bc_ps", name="bc_ps")
                    nc.tensor.matmul(ps_bc[:], ones_r[0:1, 0:128], r1row[e][0:1, tsl],
                                     start=True, stop=True)
                    bcb = lnscr.tile([128, MH], BF16, tag="bc_b", name="bc_b")
                    nc.scalar.copy(out=bcb[:], in_=ps_bc[:])
                    for kc in range(DC):
                        nc.vector.tensor_mul(out=xeT[e][kc][:, tsl],
                                             in0=ln2T[kc][:, tsl], in1=bcb[:])
            ln2T_pool.release()
            ps_r1.release()

            # ---- MoE head mms + qgelu -> ohT ----
            ps_h = pool("ps_h", 4, "PSUM")

            ohT = [ohT_pool.tile([128, NT2], BF16, tag=f"ohT_{fc}", name=f"ohT_{fc}")
                   for fc in range(FC)]
            for fc in range(FC):
                wh_tiles = []
                for e in range(E1):
                    wt_ = whp.tile([128, DC, 128], BF16, tag="wh_s", name="wh_s")
                    nc.sync.dma_start(
                        out=wt_[:],
                        in_=whT[e, :, fc * 128:(fc + 1) * 128].rearrange(
                            "(kc p) f -> p kc f", p=128))
                    wh_tiles.append(wt_)
                for hf in range(2):
                    tsl = slice(hf * MH, (hf + 1) * MH)
                    ps = ps_h.tile([128, MH], F32, tag="mh_ps", name="mh_ps")
                    first = True
                    for e in range(E1):
                        for kc in range(DC):
                            nc.tensor.matmul(ps[:], wh_tiles[e][:, kc, :], xeT[e][kc][:, tsl],
                                             start=first, stop=False)
                            first = False
                    nc.tensor.matmul(ps[:], bh_t[:, fc * 128:(fc + 1) * 128], r1nT[:, tsl],
                                     start=False, stop=True)
                    nc.scalar.activation(out=ohT[fc][:, tsl], in_=ps[:],
                                         func=AF.Gelu_apprx_sigmoid)
            ps_h.release()
            whp.release()
            xeT_pool.release()

            # ---- router 2 ----
            ps_r2 = pool("ps_r2", 2, "PSUM")
            r2nT = rper.tile([E1, NT2], BF16, name="r2nT")
            router(ps_r2, ohT, r2w_t, FC, r2b_t, r2nT)
            r2row = [rper.tile([1, NT2], BF16, tag=f"r2row_{e}", name=f"r2row_{e}")
                     for e in range(E1)]
            for e in range(E1):
                nc.sync.dma_start(out=r2row[e][0:1, :], in_=r2nT[e:e + 1, :])
            # r2 routing weights broadcast across partitions, in SBUF (bf16-exact)
            bc2 = [rper.tile([128, NT2], BF16, tag=f"bc2_{e}", name=f"bc2_{e}")
                   for e in range(E1)]
            for e in range(E1):
                for hf in range(2):
                    tsl = slice(hf * MH, (hf + 1) * MH)
                    ps_bc = ps_r2.tile([128, MH], F32, tag="bc_ps", name="bc_ps")
                    nc.tensor.matmul(ps_bc[:], ones_r[0:1, 0:128], r2row[e][0:1, tsl],
                                     start=True, stop=True)
                    nc.scalar.copy(out=bc2[e][:, tsl], in_=ps_bc[:])
            ps_r2.release()

            # ---- MoE tails (output-scaled) + residual + transpose + store ----
            accp = pool("accp", 1, side="right")
            ps_t = pool("ps_t", 2, "PSUM")

            acc = [accp.tile([128, NT2], F32, tag=f"acc_{fc2}", name=f"acc_{fc2}")
                   for fc2 in range(DC)]
            for fc2 in range(DC):
                ps_b = {}
                for hf in range(2):
                    tsl = slice(hf * MH, (hf + 1) * MH)
                    pb = ps_t.tile([128, MH], F32, tag="tl_bias", name="tl_bias")
                    nc.tensor.matmul(pb[:], bt_t[:, fc2 * 128:(fc2 + 1) * 128],
                                     r2nT[:, tsl], start=True, stop=True)
                    ps_b[hf] = pb
                for e in range(E1):
                    wtt = wtp.tile([128, FC, 128], BF16, tag="wt_s", name="wt_s")
                    nc.sync.dma_start(
                        out=wtt[:],
                        in_=wtT[e, :, fc2 * 128:(fc2 + 1) * 128].rearrange(
                            "(kc p) f -> p kc f", p=128))
                    for hf in range(2):
                        tsl = slice(hf * MH, (hf + 1) * MH)
                        ps_e = ps_t.tile([128, MH], F32, tag="tl_ps", name="tl_ps")
                        for kc in range(FC):
                            nc.tensor.matmul(ps_e[:], wtt[:, kc, :], ohT[kc][:, tsl],
                                             start=(kc == 0), stop=(kc == FC - 1))
                        tmp = lnscr.tile([128, MH], F32, tag="tl_tmp", name="tl_tmp")
                        if e == 0:
                            nc.vector.tensor_mul(out=acc[fc2][:, tsl], in0=ps_e[:],
                                                 in1=bc2[e][:, tsl])
                        else:
                            nc.vector.tensor_mul(out=tmp[:], in0=ps_e[:],
                                                 in1=bc2[e][:, tsl])
                            nc.vector.tensor_add(out=acc[fc2][:, tsl],
                                                 in0=acc[fc2][:, tsl], in1=tmp[:])
                for hf in range(2):
                    tsl = slice(hf * MH, (hf + 1) * MH)
                    nc.vector.tensor_add(out=acc[fc2][:, tsl],
                                         in0=acc[fc2][:, tsl], in1=ps_b[hf][:])

            wtp.release()
            outp = pool("outp", 1)
            ps_o = pool("ps_o", 2, "PSUM")

            # final: out = x + moe; per-fc2 so output work overlaps later tails
            ot_tiles = {}
            for c in range(CPC):
                for lc in range(2):
                    ot_tiles[(c, lc)] = outp.tile([128, D], F32,
                                                  tag=f"out_{c}_{lc}", name=f"out_{c}_{lc}")
            for fc2 in range(DC):
                for c in range(CPC):
                    for lc, (l0, nl) in enumerate(MCHUNKS):
                        tb = c * L + l0
                        ps_f = ps_o.tile([128, 128], F32, tag="out_tp", name="out_tp")
                        nc.tensor.transpose(ps_f[0:nl, :], acc[fc2][:, tb:tb + nl],
                                            id32_t[0:128, 0:128])
                        nc.vector.tensor_add(
                            out=ot_tiles[(c, lc)][0:nl, fc2 * 128:(fc2 + 1) * 128],
                            in0=xc_tok[c][lc][0:nl, fc2 * 128:(fc2 + 1) * 128],
                            in1=ps_f[0:nl, :])
            for c in range(CPC):
                for lc, (l0, nl) in enumerate(MCHUNKS):
                    nc.sync.dma_start(out=out[l0:l0 + nl, c, :],
                                      in_=ot_tiles[(c, lc)][0:nl, :])

            ps_o.release()
            ps_t.release()
            outp.release()
            accp.release()
            ohT_pool.release()
            rper.release()
            lnscr.release()
            consts.release()
            persist.release()

        for rep in range(reps):
            _body(rep)

    nc.finalize()
    return nc


def _prep_inputs(inputs):
    """Host-side: transpose/stack/cast weights, build per-core in_maps."""
    bf = ml_dtypes.bfloat16
    f32 = np.float32

    def tb(a):
        return np.ascontiguousarray(np.asarray(a, f32).T).astype(bf)

    x = np.asarray(inputs["x"], f32)              # (197, 32, 768)
    Wh = np.concatenate([np.asarray(inputs["cfc_w"], f32)[None],
                         np.asarray(inputs["eh_w"], f32)], 0)     # (5, 3072, 768)
    bh = np.concatenate([np.asarray(inputs["cfc_b"], f32)[None],
                         np.asarray(inputs["eh_b"], f32)], 0)     # (5, 3072)
    Wt = np.concatenate([np.asarray(inputs["cproj_w"], f32)[None],
                         np.asarray(inputs["et_w"], f32)], 0)     # (5, 768, 3072)
    btl = np.concatenate([np.asarray(inputs["cproj_b"], f32)[None],
                          np.asarray(inputs["et_b"], f32)], 0)    # (5, 768)

    shared = {
        "id32": np.eye(128, dtype=f32),
        "id16": np.eye(128, dtype=f32).astype(bf),
        "mfcwT": tb(inputs["msg_fc_w"]),
        "mqkvT": tb(inputs["msg_attn_wqkv"]),
        "mwoT": tb(inputs["msg_attn_wo"]),
        "wqkvT": tb(inputs["attn_wqkv"]),
        "woT": tb(inputs["attn_wo"]),
        "whT": np.ascontiguousarray(Wh.transpose(0, 2, 1)).astype(bf),
        "wtT": np.ascontiguousarray(Wt.transpose(0, 2, 1)).astype(bf),
        "r1wT": tb(inputs["r1_w"]),
        "r2wT": tb(inputs["r2_w"]),
        "bhE": bh.astype(bf),
        "btE": btl.astype(bf),
        "mfcb_r": np.asarray(inputs["msg_fc_b"], f32)[None].astype(bf),
        "mqkvb_r": np.asarray(inputs["msg_attn_bqkv"], f32)[None].astype(bf),
        "mob_r": np.asarray(inputs["msg_attn_bo"], f32)[None].astype(bf),
        "wob_r": np.asarray(inputs["attn_bo"], f32)[None].astype(bf),
        "r1b_r": np.asarray(inputs["r1_b"], f32)[None].astype(bf),
        "r2b_r": np.asarray(inputs["r2_b"], f32)[None].astype(bf),
        "qkvb": np.asarray(inputs["attn_bqkv"], f32),
        "mlng": np.asarray(inputs["msg_ln_g"], f32),
        "mlnb": np.asarray(inputs["msg_ln_b"], f32),
        "ln1g": np.asarray(inputs["ln1_g"], f32),
        "ln1b": np.asarray(inputs["ln1_b"], f32),
        "ln2g": np.asarray(inputs["ln2_g"], f32),
        "ln2b": np.asarray(inputs["ln2_b"], f32),
    }

    in_maps = []
    for core in range(NCORES):
        c0 = core * CPC
        b = c0 // T
        off = c0 - b * T
        order = list(range(off, off + CPC)) + \
            [i for i in range(T) if not (off <= i < off + CPC)]
        x0cv = x[0, b * T: (b + 1) * T, :][order]
        m = dict(shared)
        m["xs"] = np.ascontiguousarray(x[:, c0:c0 + CPC, :])
        m["x0c"] = np.ascontiguousarray(x0cv)
        in_maps.append(m)
    return in_maps


def kernel(**inputs):
    if "nc" not in _CACHE:
        _CACHE["nc"] = build()
    nc = _CACHE["nc"]
    in_maps = _prep_inputs(inputs)
    res = run_bass_kernel_spmd(nc, in_maps, core_ids=list(range(NCORES)))
    out = np.concatenate([res.results[i]["out"] for i in range(NCORES)], axis=1)
    return out.astype(np.float32)



# revision 14
# speedup vs baseline: 1.2486x; 1.2486x over previous
"""Trainium2 Bass kernel for nn_CrossFramelAttentionBlock.

Data-parallel over the bt=32 batch-columns: 8 cores x 4 columns each, no
collectives. The tiny msg (CLS cross-frame) attention mixes only the T=8
frames of one batch element b; each core computes it for its own b (8 CLS
tokens fed as a per-core input, rotated so this core's 4 columns are rows
0..3 -- attention is permutation-equivariant, so the rotation is sound).

Layout strategy: activations token-major (LN/softmax use per-partition
scalars), feature-major PE-transposed copies feed matmuls. The big GEMMs
(qkv, out-proj, MoE head, MoE tail, router2) run fp8e4 with DoubleRow
perf mode (2 contraction subtiles per instruction) accumulating in fp32
PSUM; attention internals stay bf16. Softmax without max-subtraction
(activations are O(1)); attention denominators via a ones-column appended
to V; per-head normalization applied feature-major via a ones-outer-
product broadcast of 1/den, so attention output is built directly in the
feature-major layout the out-proj consumes. MoE routing applied
input-side (head) / output-side (tail); biases via K<=5 matmuls.
"""

import numpy as np
import ml_dtypes

import concourse.bass as bass
import concourse.tile as tile
from concourse import mybir, bacc
from concourse.bass_utils import run_bass_kernel_spmd

F32 = mybir.dt.float32
BF16 = mybir.dt.bfloat16
FP8 = mybir.dt.float8e4
AF = mybir.ActivationFunctionType
OP = mybir.AluOpType
DR = mybir.MatmulPerfMode.DoubleRow

D = 768
H = 12
T = 8
E1 = 5          # E + 1 experts (shared c_fc/c_proj is expert 0)
F = 3072
L = 197
LC = 198        # L + msg token
BT = 32
NCORES = 8
CPC = 4         # columns per core
DC = 6          # 128-chunks of D
FC = 24         # 128-chunks of F
NT = CPC * LC   # 792 attention tokens per core (incl 4 msg cols at 788..791)
NT2 = CPC * L   # 788 moe tokens per core
CB = 208        # padded column block (16-aligned) for fp8 DoubleRow layouts
PNT2 = CPC * CB     # 832: padded moe-token span
PH = PNT2 // 2      # 416: half-span (2 column blocks)
PNT = PNT2 + 16     # 848: + msg tokens at 832..835
MSG0 = PNT2         # msg-token column base in ln1T8

MCHUNKS = [(0, 128), (128, 69)]   # regular-token chunks per column

_CACHE = {}


def _bcast_row_ap(dram_ap, n):
    """DRAM AP of a 1-D tensor broadcast across n partitions."""
    return bass.AP(tensor=dram_ap.tensor, offset=dram_ap.offset,
                   ap=[[0, n]] + list(dram_ap.ap))


def build(reps=1, gelu_compose=False, debug_taps=False,
          ln_trivial=()):
    nc = bacc.Bacc()

    def inp(name, shape, dt=BF16):
        return nc.declare_dram_parameter(name, list(shape), dt, isOutput=False)

    xs = inp("xs", (L, CPC, D), F32)
    x0c = inp("x0c", (T, D), F32)       # this core's b CLS rows, rotated
    id32 = inp("id32", (128, 128), F32)
    id16 = inp("id16", (128, 128), BF16)
    mfcwT = inp("mfcwT", (D, D))
    mqkvT = inp("mqkvT", (D, 3 * D))
    mwoT = inp("mwoT", (D, D))
    wqkvT8 = inp("wqkvT8", (D, 3 * D), FP8)
    woT8 = inp("woT8", (D, D), FP8)
    whT8 = inp("whT8", (E1, D, F), FP8)
    wtT8 = inp("wtT8", (E1, F, D), FP8)
    r1wT = inp("r1wT", (D, E1))
    r2wT8 = inp("r2wT8", (F, E1), FP8)
    bhE = inp("bhE", (E1, F))
    btE = inp("btE", (E1, D))
    mfcb_r = inp("mfcb_r", (1, D))
    mqkvb_r = inp("mqkvb_r", (1, 3 * D))
    mob_r = inp("mob_r", (1, D))
    wob_r = inp("wob_r", (1, D))
    r1b_r = inp("r1b_r", (1, E1))
    r2b_r = inp("r2b_r", (1, E1))
    qkvb = inp("qkvb", (3 * D,), F32)
    mlng = inp("mlng", (D,), F32)
    mlnb = inp("mlnb", (D,), F32)
    ln1g = inp("ln1g", (D,), F32)
    ln1b = inp("ln1b", (D,), F32)
    ln2g = inp("ln2g", (D,), F32)
    ln2b = inp("ln2b", (D,), F32)

    out = nc.declare_dram_parameter("out", [L, CPC, D], F32, isOutput=True)
    if debug_taps:
        dbg = {
            "dbg_qkv": nc.declare_dram_parameter("dbg_qkv", [128, NT], BF16, isOutput=True),
            "dbg_att": nc.declare_dram_parameter("dbg_att", [128, PNT2], FP8, isOutput=True),
            "dbg_ln2": nc.declare_dram_parameter("dbg_ln2", [128, PNT2], BF16, isOutput=True),
            "dbg_r1": nc.declare_dram_parameter("dbg_r1", [E1, PNT2], BF16, isOutput=True),
            "dbg_xe": nc.declare_dram_parameter("dbg_xe", [128, PNT2], FP8, isOutput=True),
            "dbg_oh": nc.declare_dram_parameter("dbg_oh", [128, PNT2], FP8, isOutput=True),
            "dbg_r2": nc.declare_dram_parameter("dbg_r2", [E1, PNT2], BF16, isOutput=True),
            "dbg_acc": nc.declare_dram_parameter("dbg_acc", [128, PNT2], F32, isOutput=True),
        }

    with tile.TileContext(nc) as tc:
        def _body(rep):
            def pool(name, bufs, space="SBUF", side=None):
                kw = {"side": side} if side else {}
                return tc.alloc_tile_pool(name=f"{name}_{rep}", bufs=bufs,
                                          space=space, **kw)

            persist = pool("persist", 1)
            consts = pool("consts", 1)
            lnscr = pool("lnscr", 2)

            # ---------------- global constants ----------------
            id32_t = consts.tile([128, 128], F32, name="id32_t")
            nc.sync.dma_start(out=id32_t[:], in_=id32[:])
            id16_t = consts.tile([128, 128], BF16, name="id16_t")
            nc.sync.dma_start(out=id16_t[:], in_=id16[:])
            ones_r = consts.tile([1, 128], BF16, name="ones_r")
            nc.vector.memset(ones_r, 1.0)
            eps_t = consts.tile([128, 1], F32, name="eps_t")
            nc.vector.memset(eps_t, 1e-5)
            qkvb_t = consts.tile([128, 18], F32, name="qkvb_t")
            nc.sync.dma_start(out=qkvb_t[:], in_=qkvb[:].rearrange("(c p) -> p c", p=128))

            def brow_tile(p, dram, n, nm):
                rt = p.tile([1, n], BF16, tag=f"brow_{nm}", name=f"brow_{nm}")
                nc.sync.dma_start(out=rt[:], in_=dram[:])
                return rt

            wob_t = brow_tile(consts, wob_r, D, "wob")
            r1b_t = brow_tile(consts, r1b_r, E1, "r1b")
            r2b_t = brow_tile(consts, r2b_r, E1, "r2b")

            def ln_params(p, g_d, b_d):
                gt = p.tile([128, D], F32, tag=f"lnp_{g_d.name}", name=f"lnp_{g_d.name}")
                nc.gpsimd.dma_start(out=gt[:], in_=_bcast_row_ap(g_d[:], 128))
                bt_ = p.tile([128, D], F32, tag=f"lnp_{b_d.name}", name=f"lnp_{b_d.name}")
                nc.gpsimd.dma_start(out=bt_[:], in_=_bcast_row_ap(b_d[:], 128))
                return gt, bt_

            # ---------------- load x (token-major, fp32) ----------------
            xc_tok = []  # [c][lc] -> [128, 768] fp32 tile
            for c in range(CPC):
                col = []
                for (l0, nl) in MCHUNKS:
                    t_ = persist.tile([128, D], F32, tag=f"xc_{c}_{l0}", name=f"xc_{c}_{l0}")
                    nc.sync.dma_start(out=t_[0:nl, :], in_=xs[l0:l0 + nl, c, :])
                    col.append(t_)
                xc_tok.append(col)

            # ---------------- helpers ----------------
            def layernorm(src_ap, nl, g_t, b_t, dst_ap):
                """token-major LN: src [nl, 768] fp32 -> dst [nl, 768] bf16"""
                stats = lnscr.tile([128, 3, 6], F32, tag="ln_stats", name="ln_stats")
                sr = src_ap.rearrange("p (c f) -> p c f", c=3)
                for cc in range(3):
                    nc.vector.bn_stats(out=stats[0:nl, cc, :], in_=sr[:, cc, :])
                mv = lnscr.tile([128, 2], F32, tag="ln_mv", name="ln_mv")
                nc.vector.bn_aggr(out=mv[0:nl, :], in_=stats[0:nl, :, :])
                rstd = lnscr.tile([128, 1], F32, tag="ln_rstd", name="ln_rstd")
                nc.scalar.activation(out=rstd[0:nl, :], in_=mv[0:nl, 1:2],
                                     func=AF.Sqrt, bias=eps_t[0:nl, :], scale=1.0)
                nc.vector.reciprocal(out=rstd[0:nl, :], in_=rstd[0:nl, :])
                if g_t is None:  # gamma==1, beta==0: normalize straight to dst
                    nc.vector.tensor_scalar(out=dst_ap, in0=src_ap,
                                            scalar1=mv[0:nl, 0:1], scalar2=rstd[0:nl, :],
                                            op0=OP.subtract, op1=OP.mult)
                    return
                tmp = lnscr.tile([128, D], F32, tag="ln_tmp", name="ln_tmp")
                nc.vector.tensor_scalar(out=tmp[0:nl, :], in0=src_ap,
                                        scalar1=mv[0:nl, 0:1], scalar2=rstd[0:nl, :],
                                        op0=OP.subtract, op1=OP.mult)
                nc.vector.tensor_mul(out=tmp[0:nl, :], in0=tmp[0:nl, :], in1=g_t[0:nl, :])
                nc.vector.tensor_add(out=dst_ap, in0=tmp[0:nl, :], in1=b_t[0:nl, :])

            def tp16(psum_pool, src_ap, np_, nf, dst_ap, tagp="tp"):
                """bf16 transpose: src [np_, nf] -> dst [nf, np_] (PE + copy)"""
                ps = psum_pool.tile([128, 128], BF16, tag=tagp, name=tagp)
                nc.tensor.transpose(ps[0:nf, 0:np_], src_ap, id16_t[0:np_, 0:np_])
                nc.any.tensor_copy(out=dst_ap, in_=ps[0:nf, 0:np_])

            # =========================================================
            # early weight prefetch (no data deps; fills DMA idle time)
            # =========================================================
            wo_pool = pool("wop", 1)
            wo_t = wo_pool.tile([128, DC, D], FP8, name="wo_t")
            nc.sync.dma_start(out=wo_t[:], in_=woT8[:].rearrange("(kc p) o -> p kc o", p=128))
            wq_pool = pool("wqp", 1)
            wq_t = [wq_pool.tile([128, DC, 128], FP8, tag=f"wq_{oc}", name=f"wq_{oc}")
                    for oc in range(18)]
            for oc in range(18):
                nc.sync.dma_start(
                    out=wq_t[oc][:],
                    in_=wqkvT8[:, oc * 128:(oc + 1) * 128].rearrange("(kc p) o -> p kc o", p=128))

            # =========================================================
            # msg path: one b, 8 CLS tokens (rows 0..3 = this core's columns)
            # =========================================================
            msgp = pool("msgp", 1)
            msgh = pool("msgh", 4)
            msps = pool("msps", 4, "PSUM")
            if "mln" in ln_trivial:
                mlng_t = mlnb_t = None
            else:
                mlng_t, mlnb_t = ln_params(msgp, mlng, mlnb)
            mfcb_t = brow_tile(msgp, mfcb_r, D, "mfcb")
            mqkvb_t = brow_tile(msgp, mqkvb_r, 3 * D, "mqkvb")
            mob_t = brow_tile(msgp, mob_r, D, "mob")

            x0_t = msgp.tile([T, D], F32, name="x0_t")
            nc.sync.dma_start(out=x0_t[:], in_=x0c[:])
            x0_b = msgp.tile([T, D], BF16, name="x0_b")
            nc.vector.tensor_copy(out=x0_b[:], in_=x0_t[:])
            x0T = msgp.tile([128, DC, T], BF16, name="x0T")
            for kc in range(DC):
                tp16(msps, x0_b[:, kc * 128:(kc + 1) * 128], T, 128, x0T[:, kc, :], "msg_ps")

            mfcw_t = msgp.tile([128, DC, D], BF16, name="mfcw_t")
            nc.sync.dma_start(out=mfcw_t[:], in_=mfcwT[:].rearrange("(kc p) o -> p kc o", p=128))

            m0 = msgp.tile([T, D], F32, name="m0")
            for oc in range(2):
                osl = slice(oc * 384, (oc + 1) * 384)
                ps = msps.tile([T, 384], F32, tag="msg_ps", name="msg_ps")
                for kc in range(DC):
                    nc.tensor.matmul(ps[:], x0T[:, kc, :], mfcw_t[:, kc, osl],
                                     start=(kc == 0), stop=False)
                nc.tensor.matmul(ps[:], ones_r[0:1, 0:T], mfcb_t[0:1, osl],
                                 start=False, stop=True)
                nc.vector.tensor_copy(out=m0[:, osl], in_=ps[:])

            mln = msgp.tile([T, D], BF16, name="mln")
            layernorm(m0[:], T, mlng_t, mlnb_t, mln[0:T, :])
            mlnT = msgp.tile([128, DC, T], BF16, name="mlnT")
            for kc in range(DC):
                tp16(msps, mln[:, kc * 128:(kc + 1) * 128], T, 128, mlnT[:, kc, :], "msg_ps")

            mqkv_t = msgp.tile([128, DC, 3 * D], BF16, name="mqkv_t")
            nc.sync.dma_start(out=mqkv_t[:], in_=mqkvT[:].rearrange("(kc p) o -> p kc o", p=128))
            qkv_m = msgp.tile([T, 3 * D], BF16, name="qkv_m")
            for oc in range(6):
                osl = slice(oc * 384, (oc + 1) * 384)
                ps = msps.tile([T, 384], F32, tag="msg_ps", name="msg_ps")
                for kc in range(DC):
                    nc.tensor.matmul(ps[:], mlnT[:, kc, :], mqkv_t[:, kc, osl],
                                     start=(kc == 0), stop=False)
                nc.tensor.matmul(ps[:], ones_r[0:1, 0:T], mqkvb_t[0:1, osl],
                                 start=False, stop=True)
                nc.vector.tensor_copy(out=qkv_m[:, osl], in_=ps[:])

            mo = msgp.tile([T, D], BF16, name="mo")
            for h in range(H):
                q_sl = qkv_m[:, h * 64:(h + 1) * 64]
                k_sl = qkv_m[:, D + h * 64: D + (h + 1) * 64]
                v_sl = qkv_m[:, 2 * D + h * 64: 2 * D + (h + 1) * 64]
                qT = msgh.tile([64, T], BF16, tag="ms_qT", name="ms_qT")
                tp16(msps, q_sl, T, 64, qT[:], "msg_ps")
                kT = msgh.tile([64, T], BF16, tag="ms_kT", name="ms_kT")
                tp16(msps, k_sl, T, 64, kT[:], "msg_ps")
                ps_s = msps.tile([T, T], F32, tag="msg_ps", name="msg_ps")
                nc.tensor.matmul(ps_s[:], qT[:], kT[:], start=True, stop=True)
                e_t = msgh.tile([T, T], BF16, tag="ms_e", name="ms_e")
                den = msgh.tile([T, 1], F32, tag="ms_den", name="ms_den")
                nc.scalar.activation(out=e_t[:], in_=ps_s[:], func=AF.Exp,
                                     scale=0.125, accum_out=den[:])
                rd = msgh.tile([T, 1], F32, tag="ms_rd", name="ms_rd")
                nc.vector.reciprocal(out=rd[:], in_=den[:])
                p_t = msgh.tile([T, T], BF16, tag="ms_p", name="ms_p")
                nc.vector.tensor_scalar_mul(out=p_t[:], in0=e_t[:], scalar1=rd[:])
                pT = msgh.tile([T, T], BF16, tag="ms_pT", name="ms_pT")
                tp16(msps, p_t[:], T, T, pT[:], "msg_ps")
                ps_o = msps.tile([T, 64], F32, tag="msg_ps", name="msg_ps")
                nc.tensor.matmul(ps_o[:], pT[:], v_sl, start=True, stop=True)
                nc.any.tensor_copy(out=mo[:, h * 64:(h + 1) * 64], in_=ps_o[:])

            moT = msgp.tile([128, DC, T], BF16, name="moT")
            for kc in range(DC):
                tp16(msps, mo[:, kc * 128:(kc + 1) * 128], T, 128, moT[:, kc, :], "msg_ps")
            mwo_t = msgp.tile([128, DC, D], BF16, name="mwo_t")
            nc.sync.dma_start(out=mwo_t[:], in_=mwoT[:].rearrange("(kc p) o -> p kc o", p=128))
            msg_tok = persist.tile([T, D], F32, name="msg_tok")
            for oc in range(2):
                osl = slice(oc * 384, (oc + 1) * 384)
                ps = msps.tile([T, 384], F32, tag="msg_ps", name="msg_ps")
                for kc in range(DC):
                    nc.tensor.matmul(ps[:], moT[:, kc, :], mwo_t[:, kc, osl],
                                     start=(kc == 0), stop=False)
                nc.tensor.matmul(ps[:], ones_r[0:1, 0:T], mob_t[0:1, osl],
                                 start=False, stop=True)
                nc.vector.tensor_add(out=msg_tok[:, osl], in0=m0[:, osl], in1=ps[:])
            msps.release()
            msgh.release()
            msgp.release()

            # =========================================================
            # LN1 (regular tokens) -> ln1T8 [128, 6, 792] fp8
            # free layout: [0:788] regular tokens (c*197+l), [788:792] msg
            # =========================================================
            ln1T_pool = pool("ln1Tp", 1)
            cln1 = pool("cln1", 1)
            if "ln1" in ln_trivial:
                ln1g_t = ln1b_t = None
            else:
                ln1g_t, ln1b_t = ln_params(cln1, ln1g, ln1b)
            ps_ln1 = pool("ps_ln1", 3, "PSUM")

            ln1T8 = ln1T_pool.tile([128, DC, PNT], FP8, name="ln1T8")
            for c in range(CPC):
                nc.vector.memset(ln1T8[:, :, c * CB + L:(c + 1) * CB], 0.0)
            nc.vector.memset(ln1T8[:, :, MSG0 + CPC:PNT], 0.0)
            for c in range(CPC):
                for lc, (l0, nl) in enumerate(MCHUNKS):
                    lnb = lnscr.tile([128, D], BF16, tag="ln_tok", name="ln_tok")
                    layernorm(xc_tok[c][lc][0:nl, :], nl, ln1g_t, ln1b_t, lnb[0:nl, :])
                    for kc in range(DC):
                        tp16(ps_ln1, lnb[0:nl, kc * 128:(kc + 1) * 128], nl, 128,
                             ln1T8[:, kc, c * CB + l0: c * CB + l0 + nl], "ln_tp")

            # =========================================================
            # main qkv pass A (788 regular tokens; no msg dependency)
            # fp8 DoubleRow; qkvT keeps the interleaved (c*198+l) layout
            # =========================================================
            qkvT_pool = pool("qkvTp", 1, side="right")
            ps_qkv = pool("ps_qkv", 4, "PSUM")

            qkvT = [qkvT_pool.tile([128, NT], BF16, tag=f"qkvT_{oc}", name=f"qkvT_{oc}")
                    for oc in range(18)]
            OC_ORDER = [tq + 6 * j for tq in range(6) for j in range(3)]
            for oc in OC_ORDER:
                for hf in range(2):
                    tsl = slice(hf * PH, (hf + 1) * PH)
                    ps = ps_qkv.tile([128, PH], F32, tag="qkv_ps", name="qkv_ps")
                    for sp in range(0, DC, 2):
                        nc.tensor.matmul(ps[:], wq_t[oc][:, sp:sp + 2, :],
                                         ln1T8[:, sp:sp + 2, tsl],
                                         start=(sp == 0), stop=(sp == DC - 2),
                                         perf_mode=DR)
                    for ci in range(2):
                        c = hf * 2 + ci
                        nc.vector.tensor_scalar_add(
                            out=qkvT[oc][:, c * LC: c * LC + L],
                            in0=ps[:, ci * CB: ci * CB + L],
                            scalar1=qkvb_t[:, oc:oc + 1])

            # ---- LN1 of the 4 msg tokens + qkv pass B ----
            lnb4 = lnscr.tile([128, D], BF16, tag="ln_tok", name="ln_tok")
            layernorm(msg_tok[0:CPC, :], CPC, ln1g_t, ln1b_t, lnb4[0:CPC, :])
            for kc in range(DC):
                tp16(ps_ln1, lnb4[0:CPC, kc * 128:(kc + 1) * 128], CPC, 128,
                     ln1T8[:, kc, MSG0:MSG0 + CPC], "ln_tp")
            for oc in OC_ORDER:
                ps = ps_qkv.tile([128, CPC], F32, tag="qkv_ps", name="qkv_ps")
                for sp in range(0, DC, 2):
                    nc.tensor.matmul(ps[:], wq_t[oc][:, sp:sp + 2, :],
                                     ln1T8[:, sp:sp + 2, MSG0:MSG0 + CPC],
                                     start=(sp == 0), stop=(sp == DC - 2),
                                     perf_mode=DR)
                nc.scalar.activation(
                    out=qkvT[oc].rearrange("p (c l) -> p c l", c=CPC)[:, :, L],
                    in_=ps[:], func=AF.Identity, bias=qkvb_t[:, oc:oc + 1], scale=1.0)
            ps_qkv.release()
            ps_ln1.release()
            cln1.release()
            ln1T_pool.release()
            wq_pool.release()

            # =========================================================
            # attention per (h, c): merged key chunks, PSUM-accumulated AV,
            # per-head 1/den broadcast -> attT8 [128, 6, 788] fp8
            # =========================================================
            attT_pool = pool("attTp", 1)
            attT8 = attT_pool.tile([128, DC, PNT2], FP8, name="attT8")
            for c in range(CPC):
                nc.vector.memset(attT8[:, :, c * CB + L:(c + 1) * CB], 0.0)
            atp = pool("atp", 3)
            oap = pool("oap", 8)
            denp = pool("denp", 2)
            v65p = pool("v65p", 1)
            psA = pool("psA", 2, "PSUM")
            psB = pool("psB", 2, "PSUM")
            psO = pool("psO", 2, "PSUM")
            psC = pool("psC", 2, "PSUM")

            v65 = [v65p.tile([128, 65], BF16, tag=f"v65_{i}", name=f"v65_{i}")
                   for i in range(4)]
            for i in range(4):
                nc.vector.memset(v65[i][:, 64:65], 1.0)
            v65_idx = 0

            KCHUNKS = [(0, 128), (128, 70)]   # key chunks incl msg token
            for h in range(H):
                tq, of = h // 2, (h % 2) * 64
                den_h = denp.tile([1, PNT2], F32, tag="den_h", name="den_h")
                for c in range(CPC):
                    nc.vector.memset(den_h[0:1, c * CB + L:(c + 1) * CB], 1.0)
                oa_c = []
                for c in range(CPC):
                    csl0 = slice(c * LC, c * LC + L)   # queries exclude msg
                    qT = qkvT[tq][of:of + 64, csl0]
                    kT = qkvT[6 + tq][of:of + 64, c * LC: (c + 1) * LC]
                    vT = qkvT[12 + tq][of:of + 64, c * LC: (c + 1) * LC]
                    ps_oa = psO.tile([65, L], F32, tag="at_oa", name="at_oa")
                    for ck, (k0, nk) in enumerate(KCHUNKS):
                        ps_s = psA.tile([128, L], F32, tag="at_s", name="at_s")
                        nc.tensor.matmul(ps_s[0:nk, :], kT[:, k0:k0 + nk], qT,
                                         start=True, stop=True)
                        e_t = atp.tile([128, L], BF16, tag="at_e", name="at_e")
                        nc.scalar.activation(out=e_t[0:nk, :], in_=ps_s[0:nk, :],
                                             func=AF.Exp, scale=0.125)
                        ps_v = psB.tile([128, 64], BF16, tag="at_vps", name="at_vps")
                        nc.tensor.transpose(ps_v[0:nk, :], vT[:, k0:k0 + nk],
                                            id16_t[of:of + 64, of:of + 64])
                        vt = v65[v65_idx % 4]
                        v65_idx += 1
                        nc.any.tensor_copy(out=vt[0:nk, 0:64], in_=ps_v[0:nk, :])
                        nc.tensor.matmul(ps_oa[:], vt[0:nk, :], e_t[0:nk, :],
                                         start=(ck == 0), stop=(ck == 1))
                    oa = oap.tile([65, L], F32, tag="at_oac", name="at_oac")
                    nc.any.tensor_copy(out=oa[:], in_=ps_oa[:])
                    nc.vector.tensor_copy(out=den_h[0:1, c * CB: c * CB + L],
                                          in_=oa[64:65, :])
                    oa_c.append(oa)
                den_r = denp.tile([1, PNT2], BF16, tag="den_r", name="den_r")
                with nc.allow_low_precision(reason="softmax 1/den in bf16 (0.4% scale)"):
                    nc.vector.reciprocal(out=den_r[0:1, :], in_=den_h[0:1, :])
                bc_ps = []
                for hf in range(2):
                    tsl = slice(hf * PH, (hf + 1) * PH)
                    pb = psC.tile([64, PH], F32, tag="at_bc", name="at_bc")
                    nc.tensor.matmul(pb[:], ones_r[0:1, 0:64], den_r[0:1, tsl],
                                     start=True, stop=True)
                    bc_ps.append(pb)
                for c in range(CPC):
                    nc.vector.tensor_mul(
                        out=attT8[of:of + 64, tq, c * CB: c * CB + L],
                        in0=oa_c[c][0:64, :],
                        in1=bc_ps[c // 2][0:64, (c % 2) * CB:(c % 2) * CB + L])
            if debug_taps:
                nc.sync.dma_start(out=dbg["dbg_qkv"][:], in_=qkvT[0][:])
            psC.release()
            psO.release()
            psB.release()
            psA.release()
            v65p.release()
            denp.release()
            oap.release()
            atp.release()
            qkvT_pool.release()

            # =========================================================
            # attention out-proj (fp8 DoubleRow) + residual into xc_tok
            # =========================================================
            ps_pr = pool("ps_pr", 4, "PSUM")
            for c in range(CPC):
                for lc, (l0, nl) in enumerate(MCHUNKS):
                    tb = c * CB + l0
                    for oc in range(2):
                        osl = slice(oc * 384, (oc + 1) * 384)
                        ps = ps_pr.tile([128, 384], F32, tag="pr_ps", name="pr_ps")
                        for sp in range(0, DC, 2):
                            nc.tensor.matmul(ps[0:nl, :],
                                             attT8[:, sp:sp + 2, tb:tb + nl],
                                             wo_t[:, sp:sp + 2, osl],
                                             start=(sp == 0), stop=False,
                                             perf_mode=DR)
                        nc.tensor.matmul(ps[0:nl, :], ones_r[0:1, 0:nl], wob_t[0:1, osl],
                                         start=False, stop=True)
                        nc.vector.tensor_add(out=xc_tok[c][lc][0:nl, osl],
                                             in0=xc_tok[c][lc][0:nl, osl], in1=ps[0:nl, :])
            if debug_taps:
                nc.sync.dma_start(out=dbg["dbg_att"][:], in_=attT8[:, 0, :])
            ps_pr.release()
            attT_pool.release()
            wo_pool.release()

            # =========================================================
            # LN2 -> ln2T [kc][128, 788] bf16
            # =========================================================
            wtp = pool("wtp", 10)
            whp = pool("whp", 30)
            ln2T_pool = pool("ln2Tp", 1)
            cln2 = pool("cln2", 1)
            if "ln2" in ln_trivial:
                ln2g_t = ln2b_t = None
            else:
                ln2g_t, ln2b_t = ln_params(cln2, ln2g, ln2b)
            ps_ln2 = pool("ps_ln2", 6, "PSUM")

            ln2T = [ln2T_pool.tile([128, PNT2], BF16, tag=f"ln2T_{kc}", name=f"ln2T_{kc}")
                    for kc in range(DC)]
            for kc in range(DC):
                for c in range(CPC):
                    nc.vector.memset(ln2T[kc][:, c * CB + L:(c + 1) * CB], 0.0)
            for c in range(CPC):
                for lc, (l0, nl) in enumerate(MCHUNKS):
                    lnb = lnscr.tile([128, D], BF16, tag="ln_tok", name="ln_tok")
                    layernorm(xc_tok[c][lc][0:nl, :], nl, ln2g_t, ln2b_t, lnb[0:nl, :])
                    for kc in range(DC):
                        tp16(ps_ln2, lnb[0:nl, kc * 128:(kc + 1) * 128], nl, 128,
                             ln2T[kc][:, c * CB + l0: c * CB + l0 + nl], "ln_tp")
            ps_ln2.release()
            cln2.release()

            # =========================================================
            # routers + MoE
            # =========================================================
            rper = pool("rper", 1, side="right")
            oh_pool = pool("ohp", 1, side="right")
            r1w_t = rper.tile([128, DC, E1], BF16, name="r1w_t")
            nc.sync.dma_start(out=r1w_t[:], in_=r1wT[:].rearrange("(kc p) e -> p kc e", p=128))
            r2w_t = rper.tile([128, FC, 16], FP8, name="r2w_t")
            nc.vector.memset(r2w_t[:, :, E1:16], 0.0)
            nc.sync.dma_start(out=r2w_t[:, :, 0:E1],
                              in_=r2wT8[:].rearrange("(kc p) e -> p kc e", p=128))
            bh_t = rper.tile([E1, F], BF16, name="bh_t")
            nc.sync.dma_start(out=bh_t[:], in_=bhE[:])
            bt_t = rper.tile([E1, D], BF16, name="bt_t")
            nc.sync.dma_start(out=bt_t[:], in_=btE[:])

            def router_softmax(ps_r, ps, nl, tb, bias_row, dstT):
                nc.tensor.matmul(ps[0:nl, :], ones_r[0:1, 0:nl], bias_row[0:1, :],
                                 start=False, stop=True)
                er = lnscr.tile([128, E1], BF16, tag="r_e", name="r_e")
                den = lnscr.tile([128, 1], F32, tag="r_den", name="r_den")
                nc.scalar.activation(out=er[0:nl, :], in_=ps[0:nl, :],
                                     func=AF.Exp, accum_out=den[0:nl, :])
                rdd = lnscr.tile([128, 1], F32, tag="r_rd", name="r_rd")
                nc.vector.reciprocal(out=rdd[0:nl, :], in_=den[0:nl, :])
                rn = lnscr.tile([128, E1], BF16, tag="r_n", name="r_n")
                nc.vector.tensor_scalar_mul(out=rn[0:nl, :], in0=er[0:nl, :],
                                            scalar1=rdd[0:nl, :])
                tp16(ps_r, rn[0:nl, :], nl, E1, dstT[0:E1, tb:tb + nl], "r_tp")

            # ---- router 1 + xeT ----
            ps_r1 = pool("ps_r1", 2, "PSUM")
            r1nT = rper.tile([E1, PNT2], BF16, name="r1nT")
            for c in range(CPC):
                nc.vector.memset(r1nT[0:E1, c * CB + L:(c + 1) * CB], 0.0)
            for c in range(CPC):
                for lc, (l0, nl) in enumerate(MCHUNKS):
                    tb = c * CB + l0
                    ps = ps_r1.tile([128, E1], F32, tag="r_ps", name="r_ps")
                    for kc in range(DC):
                        nc.tensor.matmul(ps[0:nl, :], ln2T[kc][:, tb:tb + nl],
                                         r1w_t[:, kc, :], start=(kc == 0), stop=False)
                    router_softmax(ps_r1, ps, nl, tb, r1b_t, r1nT)
            r1row = [rper.tile([1, PNT2], BF16, tag=f"r1row_{e}", name=f"r1row_{e}")
                     for e in range(E1)]
            for e in range(E1):
                nc.sync.dma_start(out=r1row[e][0:1, :], in_=r1nT[e:e + 1, :])

            xeT_pool = pool("xeTp", 1, side="right")
            xeT = [xeT_pool.tile([128, DC, PNT2], FP8, tag=f"xeT_{e}", name=f"xeT_{e}")
                   for e in range(E1)]
            for e in range(E1):
                for hf in range(2):
                    tsl = slice(hf * PH, (hf + 1) * PH)
                    ps_bc = ps_r1.tile([128, PH], F32, tag="bc_ps", name="bc_ps")
                    nc.tensor.matmul(ps_bc[:], ones_r[0:1, 0:128], r1row[e][0:1, tsl],
                                     start=True, stop=True)
                    bcb = lnscr.tile([128, PH], BF16, tag="bc_b", name="bc_b")
                    nc.scalar.copy(out=bcb[:], in_=ps_bc[:])
                    for kc in range(DC):
                        nc.vector.tensor_mul(out=xeT[e][:, kc, tsl],
                                             in0=ln2T[kc][:, tsl], in1=bcb[:])
            if debug_taps:
                nc.sync.dma_start(out=dbg["dbg_ln2"][:], in_=ln2T[0][:])
                nc.sync.dma_start(out=dbg["dbg_r1"][:], in_=r1nT[0:E1, :])
                nc.sync.dma_start(out=dbg["dbg_xe"][:], in_=xeT[0][:, 0, :])
            ln2T_pool.release()
            ps_r1.release()

            # ---- MoE head mms (fp8 DoubleRow) + qgelu -> oh8 ----
            # router2 logits accumulate incrementally as oh8 fc-pairs complete
            ps_r2a = pool("ps_r2a", 1, "PSUM")
            ps_h = pool("ps_h", 4, "PSUM")
            r2all = ps_r2a.tile([128, 8, 8], F32, name="r2all")
            oh8 = oh_pool.tile([128, FC, PNT2], FP8, name="oh8")
            for fc in range(FC):
                wh_tiles = []
                for e in range(E1):
                    wt_ = whp.tile([128, DC, 128], FP8, tag="wh_s", name="wh_s")
                    nc.sync.dma_start(
                        out=wt_[:],
                        in_=whT8[e, :, fc * 128:(fc + 1) * 128].rearrange(
                            "(kc p) f -> p kc f", p=128))
                    wh_tiles.append(wt_)
                for hf in range(2):
                    tsl = slice(hf * PH, (hf + 1) * PH)
                    ps = ps_h.tile([128, PH], F32, tag="mh_ps", name="mh_ps")
                    first = True
                    for e in range(E1):
                        for sp in range(0, DC, 2):
                            nc.tensor.matmul(ps[:], wh_tiles[e][:, sp:sp + 2, :],
                                             xeT[e][:, sp:sp + 2, tsl],
                                             start=first, stop=False, perf_mode=DR)
                            first = False
                    nc.tensor.matmul(ps[:], bh_t[:, fc * 128:(fc + 1) * 128], r1nT[:, tsl],
                                     start=False, stop=True)
                    if gelu_compose:
                        sg = lnscr.tile([128, PH], BF16, tag="sg", name="sg")
                        nc.scalar.activation(out=sg[:], in_=ps[:],
                                             func=AF.Sigmoid, scale=1.702)
                        nc.vector.tensor_mul(out=oh8[:, fc, tsl], in0=ps[:], in1=sg[:])
                    else:
                        nc.scalar.activation(out=oh8[:, fc, tsl], in_=ps[:],
                                             func=AF.Gelu_apprx_sigmoid)
            if debug_taps:
                nc.sync.dma_start(out=dbg["dbg_oh"][:], in_=oh8[:, 0, :])
            ps_h.release()
            whp.release()
            xeT_pool.release()

            # ---- router 2 softmax (logits already accumulated in r2all) ----
            ps_r2 = pool("ps_r2", 2, "PSUM")
            r2nT = rper.tile([E1, PNT2], BF16, name="r2nT")
            for c in range(CPC):
                nc.vector.memset(r2nT[0:E1, c * CB + L:(c + 1) * CB], 0.0)
            for ch, (c, (l0, nl)) in enumerate(
                    (c, m) for c in range(CPC) for m in MCHUNKS):
                tb = c * CB + l0
                router_softmax(ps_r2, r2all[:, ch, 0:E1], nl, tb, r2b_t, r2nT)
            r2row = [rper.tile([1, PNT2], BF16, tag=f"r2row_{e}", name=f"r2row_{e}")
                     for e in range(E1)]
            for e in range(E1):
                nc.sync.dma_start(out=r2row[e][0:1, :], in_=r2nT[e:e + 1, :])
            # r2 routing weights broadcast across partitions, in SBUF (bf16-exact)
            bc2 = [rper.tile([128, PNT2], BF16, tag=f"bc2_{e}", name=f"bc2_{e}")
                   for e in range(E1)]
            for e in range(E1):
                for hf in range(2):
                    tsl = slice(hf * PH, (hf + 1) * PH)
                    ps_bc = ps_r2.tile([128, PH], F32, tag="bc_ps", name="bc_ps")
                    nc.tensor.matmul(ps_bc[:], ones_r[0:1, 0:128], r2row[e][0:1, tsl],
                                     start=True, stop=True)
                    nc.scalar.copy(out=bc2[e][:, tsl], in_=ps_bc[:])
            ps_r2.release()
            ps_r2a.release()

            # ---- MoE tails (fp8 DoubleRow, output-scaled) + residual + store ----
            accp = pool("accp", 1, side="right")
            ps_t = pool("ps_t", 2, "PSUM")

            acc = [accp.tile([128, PNT2], F32, tag=f"acc_{fc2}", name=f"acc_{fc2}")
                   for fc2 in range(DC)]
            for fc2 in range(DC):
                ps_b = {}
                for hf in range(2):
                    tsl = slice(hf * PH, (hf + 1) * PH)
                    pb = ps_t.tile([128, PH], F32, tag="tl_bias", name="tl_bias")
                    nc.tensor.matmul(pb[:], bt_t[:, fc2 * 128:(fc2 + 1) * 128],
                                     r2nT[:, tsl], start=True, stop=True)
                    ps_b[hf] = pb
                for e in range(E1):
                    wtt = wtp.tile([128, FC, 128], FP8, tag="wt_s", name="wt_s")
                    nc.sync.dma_start(
                        out=wtt[:],
                        in_=wtT8[e, :, fc2 * 128:(fc2 + 1) * 128].rearrange(
                            "(kc p) f -> p kc f", p=128))
                    for hf in range(2):
                        tsl = slice(hf * PH, (hf + 1) * PH)
                        ps_e = ps_t.tile([128, PH], F32, tag="tl_ps", name="tl_ps")
                        for sp in range(0, FC, 2):
                            nc.tensor.matmul(ps_e[:], wtt[:, sp:sp + 2, :],
                                             oh8[:, sp:sp + 2, tsl],
                                             start=(sp == 0), stop=(sp == FC - 2),
                                             perf_mode=DR)
                        tmp = lnscr.tile([128, PH], F32, tag="tl_tmp", name="tl_tmp")
                        if e == 0:
                            nc.vector.tensor_mul(out=acc[fc2][:, tsl], in0=ps_e[:],
                                                 in1=bc2[e][:, tsl])
                        else:
                            nc.vector.tensor_mul(out=tmp[:], in0=ps_e[:],
                                                 in1=bc2[e][:, tsl])
                            nc.gpsimd.tensor_add(out=acc[fc2][:, tsl],
                                                 in0=acc[fc2][:, tsl], in1=tmp[:])
                for hf in range(2):
                    tsl = slice(hf * PH, (hf + 1) * PH)
                    nc.vector.tensor_add(out=acc[fc2][:, tsl],
                                         in0=acc[fc2][:, tsl], in1=ps_b[hf][:])

            if debug_taps:
                nc.sync.dma_start(out=dbg["dbg_r2"][:], in_=r2nT[0:E1, :])
                nc.sync.dma_start(out=dbg["dbg_acc"][:], in_=acc[0][:])
            wtp.release()
            outp = pool("outp", 1)
            ps_o = pool("ps_o", 2, "PSUM")

            # final: out = x + moe; per-fc2 so output work overlaps later tails
            ot_tiles = {}
            for c in range(CPC):
                for lc in range(2):
                    ot_tiles[(c, lc)] = outp.tile([128, D], F32,
                                                  tag=f"out_{c}_{lc}", name=f"out_{c}_{lc}")
            for fc2 in range(DC):
                for c in range(CPC):
                    for lc, (l0, nl) in enumerate(MCHUNKS):
                        tb = c * CB + l0
                        ps_f = ps_o.tile([128, 128], F32, tag="out_tp", name="out_tp")
                        nc.tensor.transpose(ps_f[0:nl, :], acc[fc2][:, tb:tb + nl],
                                            id32_t[0:128, 0:128])
                        nc.vector.tensor_add(
                            out=ot_tiles[(c, lc)][0:nl, fc2 * 128:(fc2 + 1) * 128],
                            in0=xc_tok[c][lc][0:nl, fc2 * 128:(fc2 + 1) * 128],
                            in1=ps_f[0:nl, :])
            for c in range(CPC):
                for lc, (l0, nl) in enumerate(MCHUNKS):
                    nc.sync.dma_start(out=out[l0:l0 + nl, c, :],
                                      in_=ot_tiles[(c, lc)][0:nl, :])

            ps_o.release()
            ps_t.release()
            outp.release()
            accp.release()
            oh_pool.release()
            rper.release()
            lnscr.release()
            consts.release()
            persist.release()

        for rep in range(reps):
            _body(rep)

    nc.finalize()
    return nc


def _prep_inputs(inputs):
    """Host-side: transpose/stack/cast weights, build per-core in_maps."""
    bf = ml_dtypes.bfloat16
    f8 = ml_dtypes.float8_e4m3
    f32 = np.float32

    def tb(a):
        return np.ascontiguousarray(np.asarray(a, f32).T).astype(bf)

    def t8(a):
        return np.ascontiguousarray(np.asarray(a, f32).T).astype(f8)

    x = np.asarray(inputs["x"], f32)              # (197, 32, 768)
    Wh = np.concatenate([np.asarray(inputs["cfc_w"], f32)[None],
                         np.asarray(inputs["eh_w"], f32)], 0)     # (5, 3072, 768)
    bh = np.concatenate([np.asarray(inputs["cfc_b"], f32)[None],
                         np.asarray(inputs["eh_b"], f32)], 0)     # (5, 3072)
    Wt = np.concatenate([np.asarray(inputs["cproj_w"], f32)[None],
                         np.asarray(inputs["et_w"], f32)], 0)     # (5, 768, 3072)
    btl = np.concatenate([np.asarray(inputs["cproj_b"], f32)[None],
                          np.asarray(inputs["et_b"], f32)], 0)    # (5, 768)

    shared = {
        "id32": np.eye(128, dtype=f32),
        "id16": np.eye(128, dtype=f32).astype(bf),
        "mfcwT": tb(inputs["msg_fc_w"]),
        "mqkvT": tb(inputs["msg_attn_wqkv"]),
        "mwoT": tb(inputs["msg_attn_wo"]),
        "wqkvT8": t8(inputs["attn_wqkv"]),
        "woT8": t8(inputs["attn_wo"]),
        "whT8": np.ascontiguousarray(Wh.transpose(0, 2, 1)).astype(f8),
        "wtT8": np.ascontiguousarray(Wt.transpose(0, 2, 1)).astype(f8),
        "r1wT": tb(inputs["r1_w"]),
        "r2wT8": t8(inputs["r2_w"]),
        "bhE": bh.astype(bf),
        "btE": btl.astype(bf),
        "mfcb_r": np.asarray(inputs["msg_fc_b"], f32)[None].astype(bf),
        "mqkvb_r": np.asarray(inputs["msg_attn_bqkv"], f32)[None].astype(bf),
        "mob_r": np.asarray(inputs["msg_attn_bo"], f32)[None].astype(bf),
        "wob_r": np.asarray(inputs["attn_bo"], f32)[None].astype(bf),
        "r1b_r": np.asarray(inputs["r1_b"], f32)[None].astype(bf),
        "r2b_r": np.asarray(inputs["r2_b"], f32)[None].astype(bf),
        "qkvb": np.asarray(inputs["attn_bqkv"], f32),
        "mlng": np.asarray(inputs["msg_ln_g"], f32),
        "mlnb": np.asarray(inputs["msg_ln_b"], f32),
        "ln1g": np.asarray(inputs["ln1_g"], f32),
        "ln1b": np.asarray(inputs["ln1_b"], f32),
        "ln2g": np.asarray(inputs["ln2_g"], f32),
        "ln2b": np.asarray(inputs["ln2_b"], f32),
    }

    in_maps = []
    for core in range(NCORES):
        c0 = core * CPC
        b = c0 // T
        off = c0 - b * T
        order = list(range(off, off + CPC)) + \
            [i for i in range(T) if not (off <= i < off + CPC)]
        x0cv = x[0, b * T: (b + 1) * T, :][order]
        m = dict(shared)
        m["xs"] = np.ascontiguousarray(x[:, c0:c0 + CPC, :])
        m["x0c"] = np.ascontiguousarray(x0cv)
        in_maps.append(m)
    return in_maps


def _ln_trivial_flags(inputs):
    flags = []
    checks = {"mln": ("msg_ln_g", "msg_ln_b"), "ln1": ("ln1_g", "ln1_b"),
              "ln2": ("ln2_g", "ln2_b")}
    for nm, (g, b) in checks.items():
        if np.allclose(np.asarray(inputs[g]), 1.0) and \
                np.allclose(np.asarray(inputs[b]), 0.0):
            flags.append(nm)
    return tuple(sorted(flags))


def kernel(**inputs):
    flags = _ln_trivial_flags(inputs)
    key = ("nc", flags)
    if key not in _CACHE:
        _CACHE[key] = build(ln_trivial=flags)
        _CACHE["nc"] = _CACHE[key]
    nc = _CACHE[key]
    expected_in = set()
    for alloc in nc.m.functions[0].allocations:
        if isinstance(alloc, mybir.MemoryLocationSet) and alloc.kind == "ExternalInput":
            expected_in.add(alloc.memorylocations[0].name)
    in_maps = [{k: v for k, v in m.items() if k in expected_in}
               for m in _prep_inputs(inputs)]
    res = run_bass_kernel_spmd(nc, in_maps, core_ids=list(range(NCORES)))
    out = np.concatenate([res.results[i]["out"] for i in range(NCORES)], axis=1)
    return out.astype(np.float32)


# revision 19
# speedup vs baseline: 5.2030x; 4.1670x over previous
"""Trainium2 Bass kernel for nn_CrossFramelAttentionBlock.

Data-parallel over the bt=32 batch-columns: 8 cores x 4 columns each, no
collectives. The tiny msg (CLS cross-frame) attention mixes only the T=8
frames of one batch element b; each core computes it for its own b (8 CLS
tokens fed as a per-core input, rotated so this core's 4 columns are rows
0..3 -- attention is permutation-equivariant, so the rotation is sound).

Layout strategy: activations token-major (LN/softmax use per-partition
scalars), feature-major PE-transposed copies feed matmuls. The big GEMMs
(qkv, out-proj, MoE head, MoE tail, router2) run fp8e4 with DoubleRow
perf mode (2 contraction subtiles per instruction) accumulating in fp32
PSUM; attention internals stay bf16. Softmax without max-subtraction
(activations are O(1)); attention denominators via a ones-column appended
to V; per-head normalization applied feature-major via a ones-outer-
product broadcast of 1/den, so attention output is built directly in the
feature-major layout the out-proj consumes. MoE routing applied
input-side (head) / output-side (tail); biases via K<=5 matmuls.
"""

import numpy as np
import ml_dtypes

import concourse.bass as bass
import concourse.tile as tile
from concourse import mybir, bacc
from concourse.bass_utils import run_bass_kernel_spmd

F32 = mybir.dt.float32
BF16 = mybir.dt.bfloat16
FP8 = mybir.dt.float8e4
AF = mybir.ActivationFunctionType
OP = mybir.AluOpType
DR = mybir.MatmulPerfMode.DoubleRow

D = 768
H = 12
T = 8
E1 = 5          # E + 1 experts (shared c_fc/c_proj is expert 0)
F = 3072
L = 197
LC = 198        # L + msg token
BT = 32
NCORES = 8
CPC = 4         # columns per core
DC = 6          # 128-chunks of D
FC = 24         # 128-chunks of F
NT = CPC * LC   # 792 attention tokens per core (incl 4 msg cols at 788..791)
NT2 = CPC * L   # 788 moe tokens per core
CB = 208        # padded column block (16-aligned) for fp8 DoubleRow layouts
PNT2 = CPC * CB     # 832: padded moe-token span
PH = PNT2 // 2      # 416: half-span (2 column blocks)
PNT = PNT2 + 16     # 848: + msg tokens at 832..835
MSG0 = PNT2         # msg-token column base in ln1T8

MCHUNKS = [(0, 128), (128, 69)]   # regular-token chunks per column

_CACHE = {}


def _bcast_row_ap(dram_ap, n):
    """DRAM AP of a 1-D tensor broadcast across n partitions."""
    return bass.AP(tensor=dram_ap.tensor, offset=dram_ap.offset,
                   ap=[[0, n]] + list(dram_ap.ap))


def build(reps=1, gelu_compose=False, debug_taps=False,
          ln_trivial=(), loop_n=0):
    nc = bacc.Bacc()

    def inp(name, shape, dt=BF16):
        return nc.declare_dram_parameter(name, list(shape), dt, isOutput=False)

    xs = inp("xs", (L, CPC, D), F32)
    x0c = inp("x0c", (T, D), F32)       # this core's b CLS rows, rotated
    id32 = inp("id32", (128, 128), F32)
    id16 = inp("id16", (128, 128), BF16)
    mfcwh = inp("mfcwh", (128, DC, D))
    mqkvh = inp("mqkvh", (128, DC, 3 * D))
    mwoh = inp("mwoh", (128, DC, D))
    wqh = inp("wqh", (18, 128, DC, 128), FP8)
    woh = inp("woh", (128, DC, D), FP8)
    whh = inp("whh", (E1, FC, 128, DC, 128), FP8)
    wth = inp("wth", (E1, DC, 128, FC, 128), FP8)
    r1wh = inp("r1wh", (128, DC, E1))
    r2wh = inp("r2wh", (128, FC, E1), FP8)
    bhE = inp("bhE", (E1, F))
    btE = inp("btE", (E1, D))
    mfcb_r = inp("mfcb_r", (1, D))
    mqkvb_r = inp("mqkvb_r", (1, 3 * D))
    mob_r = inp("mob_r", (1, D))
    wob_r = inp("wob_r", (1, D))
    r1b_r = inp("r1b_r", (1, E1))
    r2b_r = inp("r2b_r", (1, E1))
    qkvb = inp("qkvb", (3 * D,), F32)
    mlng = inp("mlng", (D,), F32)
    mlnb = inp("mlnb", (D,), F32)
    ln1g = inp("ln1g", (D,), F32)
    ln1b = inp("ln1b", (D,), F32)
    ln2g = inp("ln2g", (D,), F32)
    ln2b = inp("ln2b", (D,), F32)

    out = nc.declare_dram_parameter("out", [L, CPC, D], F32, isOutput=True)
    if debug_taps:
        dbg = {
            "dbg_qkv": nc.declare_dram_parameter("dbg_qkv", [128, NT], BF16, isOutput=True),
            "dbg_att": nc.declare_dram_parameter("dbg_att", [128, PNT2], FP8, isOutput=True),
            "dbg_ln2": nc.declare_dram_parameter("dbg_ln2", [128, PNT2], BF16, isOutput=True),
            "dbg_r1": nc.declare_dram_parameter("dbg_r1", [E1, PNT2], BF16, isOutput=True),
            "dbg_xe": nc.declare_dram_parameter("dbg_xe", [128, PNT2], FP8, isOutput=True),
            "dbg_oh": nc.declare_dram_parameter("dbg_oh", [128, PNT2], FP8, isOutput=True),
            "dbg_r2": nc.declare_dram_parameter("dbg_r2", [E1, PNT2], BF16, isOutput=True),
            "dbg_acc": nc.declare_dram_parameter("dbg_acc", [128, PNT2], F32, isOutput=True),
        }

    with tile.TileContext(nc) as tc:
        def _body(rep):
            def pool(name, bufs, space="SBUF", side=None):
                kw = {"side": side} if side else {}
                return tc.alloc_tile_pool(name=f"{name}_{rep}", bufs=bufs,
                                          space=space, **kw)

            persist = pool("persist", 1)
            consts = pool("consts", 1)
            lnscr = pool("lnscr", 2)

            # ---------------- global constants ----------------
            id32_t = consts.tile([128, 128], F32, name="id32_t")
            nc.sync.dma_start(out=id32_t[:], in_=id32[:])
            id16_t = consts.tile([128, 128], BF16, name="id16_t")
            nc.sync.dma_start(out=id16_t[:], in_=id16[:])
            ones_r = consts.tile([1, 128], BF16, name="ones_r")
            nc.vector.memset(ones_r, 1.0)
            eps_t = consts.tile([128, 1], F32, name="eps_t")
            nc.vector.memset(eps_t, 1e-5)
            qkvb_t = consts.tile([128, 18], F32, name="qkvb_t")
            nc.sync.dma_start(out=qkvb_t[:], in_=qkvb[:].rearrange("(c p) -> p c", p=128))

            def brow_tile(p, dram, n, nm):
                rt = p.tile([1, n], BF16, tag=f"brow_{nm}", name=f"brow_{nm}")
                nc.sync.dma_start(out=rt[:], in_=dram[:])
                return rt

            wob_t = brow_tile(consts, wob_r, D, "wob")
            r1b_t = brow_tile(consts, r1b_r, E1, "r1b")
            r2b_t = brow_tile(consts, r2b_r, E1, "r2b")

            def ln_params(p, g_d, b_d):
                gt = p.tile([128, D], F32, tag=f"lnp_{g_d.name}", name=f"lnp_{g_d.name}")
                nc.gpsimd.dma_start(out=gt[:], in_=_bcast_row_ap(g_d[:], 128))
                bt_ = p.tile([128, D], F32, tag=f"lnp_{b_d.name}", name=f"lnp_{b_d.name}")
                nc.gpsimd.dma_start(out=bt_[:], in_=_bcast_row_ap(b_d[:], 128))
                return gt, bt_

            # ---------------- load x (token-major, fp32) ----------------
            xc_tok = []  # [c][lc] -> [128, 768] fp32 tile
            for c in range(CPC):
                col = []
                for (l0, nl) in MCHUNKS:
                    t_ = persist.tile([128, D], F32, tag=f"xc_{c}_{l0}", name=f"xc_{c}_{l0}")
                    nc.sync.dma_start(out=t_[0:nl, :], in_=xs[l0:l0 + nl, c, :])
                    col.append(t_)
                xc_tok.append(col)

            # ---------------- helpers ----------------
            def layernorm(src_ap, nl, g_t, b_t, dst_ap):
                """token-major LN: src [nl, 768] fp32 -> dst [nl, 768] bf16"""
                stats = lnscr.tile([128, 3, 6], F32, tag="ln_stats", name="ln_stats")
                sr = src_ap.rearrange("p (c f) -> p c f", c=3)
                for cc in range(3):
                    nc.vector.bn_stats(out=stats[0:nl, cc, :], in_=sr[:, cc, :])
                mv = lnscr.tile([128, 2], F32, tag="ln_mv", name="ln_mv")
                nc.vector.bn_aggr(out=mv[0:nl, :], in_=stats[0:nl, :, :])
                rstd = lnscr.tile([128, 1], F32, tag="ln_rstd", name="ln_rstd")
                nc.scalar.activation(out=rstd[0:nl, :], in_=mv[0:nl, 1:2],
                                     func=AF.Sqrt, bias=eps_t[0:nl, :], scale=1.0)
                nc.vector.reciprocal(out=rstd[0:nl, :], in_=rstd[0:nl, :])
                if g_t is None:  # gamma==1, beta==0: normalize straight to dst
                    nc.vector.tensor_scalar(out=dst_ap, in0=src_ap,
                                            scalar1=mv[0:nl, 0:1], scalar2=rstd[0:nl, :],
                                            op0=OP.subtract, op1=OP.mult)
                    return
                tmp = lnscr.tile([128, D], F32, tag="ln_tmp", name="ln_tmp")
                nc.vector.tensor_scalar(out=tmp[0:nl, :], in0=src_ap,
                                        scalar1=mv[0:nl, 0:1], scalar2=rstd[0:nl, :],
                                        op0=OP.subtract, op1=OP.mult)
                nc.vector.tensor_mul(out=tmp[0:nl, :], in0=tmp[0:nl, :], in1=g_t[0:nl, :])
                nc.vector.tensor_add(out=dst_ap, in0=tmp[0:nl, :], in1=b_t[0:nl, :])

            def tp16(psum_pool, src_ap, np_, nf, dst_ap, tagp="tp"):
                """bf16 transpose: src [np_, nf] -> dst [nf, np_] (PE + copy)"""
                ps = psum_pool.tile([128, 128], BF16, tag=tagp, name=tagp)
                nc.tensor.transpose(ps[0:nf, 0:np_], src_ap, id16_t[0:np_, 0:np_])
                nc.any.tensor_copy(out=dst_ap, in_=ps[0:nf, 0:np_])

            # =========================================================
            # early weight prefetch (no data deps; fills DMA idle time)
            # =========================================================
            wo_pool = pool("wop", 1)
            wo_t = wo_pool.tile([128, DC, D], FP8, name="wo_t")
            nc.sync.dma_start(out=wo_t[:], in_=woh[:])
            wq_pool = pool("wqp", 1)
            wq_t = [wq_pool.tile([128, DC, 128], FP8, tag=f"wq_{oc}", name=f"wq_{oc}")
                    for oc in range(18)]
            for oc in range(18):
                nc.sync.dma_start(out=wq_t[oc][:], in_=wqh[oc])

            # =========================================================
            # msg path: one b, 8 CLS tokens (rows 0..3 = this core's columns)
            # =========================================================
            msgp = pool("msgp", 1)
            msgh = pool("msgh", 4)
            msps = pool("msps", 4, "PSUM")
            if "mln" in ln_trivial:
                mlng_t = mlnb_t = None
            else:
                mlng_t, mlnb_t = ln_params(msgp, mlng, mlnb)
            mfcb_t = brow_tile(msgp, mfcb_r, D, "mfcb")
            mqkvb_t = brow_tile(msgp, mqkvb_r, 3 * D, "mqkvb")
            mob_t = brow_tile(msgp, mob_r, D, "mob")

            x0_t = msgp.tile([T, D], F32, name="x0_t")
            nc.sync.dma_start(out=x0_t[:], in_=x0c[:])
            x0_b = msgp.tile([T, D], BF16, name="x0_b")
            nc.vector.tensor_copy(out=x0_b[:], in_=x0_t[:])
            x0T = msgp.tile([128, DC, T], BF16, name="x0T")
            for kc in range(DC):
                tp16(msps, x0_b[:, kc * 128:(kc + 1) * 128], T, 128, x0T[:, kc, :], "msg_ps")

            mfcw_t = msgp.tile([128, DC, D], BF16, name="mfcw_t")
            nc.sync.dma_start(out=mfcw_t[:], in_=mfcwh[:])

            m0 = msgp.tile([T, D], F32, name="m0")
            for oc in range(2):
                osl = slice(oc * 384, (oc + 1) * 384)
                ps = msps.tile([T, 384], F32, tag="msg_ps", name="msg_ps")
                for kc in range(DC):
                    nc.tensor.matmul(ps[:], x0T[:, kc, :], mfcw_t[:, kc, osl],
                                     start=(kc == 0), stop=False)
                nc.tensor.matmul(ps[:], ones_r[0:1, 0:T], mfcb_t[0:1, osl],
                                 start=False, stop=True)
                nc.vector.tensor_copy(out=m0[:, osl], in_=ps[:])

            mln = msgp.tile([T, D], BF16, name="mln")
            layernorm(m0[:], T, mlng_t, mlnb_t, mln[0:T, :])
            mlnT = msgp.tile([128, DC, T], BF16, name="mlnT")
            for kc in range(DC):
                tp16(msps, mln[:, kc * 128:(kc + 1) * 128], T, 128, mlnT[:, kc, :], "msg_ps")

            mqkv_t = msgp.tile([128, DC, 3 * D], BF16, name="mqkv_t")
            nc.sync.dma_start(out=mqkv_t[:], in_=mqkvh[:])
            qkv_m = msgp.tile([T, 3 * D], BF16, name="qkv_m")
            for oc in range(6):
                osl = slice(oc * 384, (oc + 1) * 384)
                ps = msps.tile([T, 384], F32, tag="msg_ps", name="msg_ps")
                for kc in range(DC):
                    nc.tensor.matmul(ps[:], mlnT[:, kc, :], mqkv_t[:, kc, osl],
                                     start=(kc == 0), stop=False)
                nc.tensor.matmul(ps[:], ones_r[0:1, 0:T], mqkvb_t[0:1, osl],
                                 start=False, stop=True)
                nc.vector.tensor_copy(out=qkv_m[:, osl], in_=ps[:])

            mo = msgp.tile([T, D], BF16, name="mo")
            for h in range(H):
                q_sl = qkv_m[:, h * 64:(h + 1) * 64]
                k_sl = qkv_m[:, D + h * 64: D + (h + 1) * 64]
                v_sl = qkv_m[:, 2 * D + h * 64: 2 * D + (h + 1) * 64]
                qT = msgh.tile([64, T], BF16, tag="ms_qT", name="ms_qT")
                tp16(msps, q_sl, T, 64, qT[:], "msg_ps")
                kT = msgh.tile([64, T], BF16, tag="ms_kT", name="ms_kT")
                tp16(msps, k_sl, T, 64, kT[:], "msg_ps")
                ps_s = msps.tile([T, T], F32, tag="msg_ps", name="msg_ps")
                nc.tensor.matmul(ps_s[:], qT[:], kT[:], start=True, stop=True)
                e_t = msgh.tile([T, T], BF16, tag="ms_e", name="ms_e")
                den = msgh.tile([T, 1], F32, tag="ms_den", name="ms_den")
                nc.scalar.activation(out=e_t[:], in_=ps_s[:], func=AF.Exp,
                                     scale=0.125, accum_out=den[:])
                rd = msgh.tile([T, 1], F32, tag="ms_rd", name="ms_rd")
                nc.vector.reciprocal(out=rd[:], in_=den[:])
                p_t = msgh.tile([T, T], BF16, tag="ms_p", name="ms_p")
                nc.vector.tensor_scalar_mul(out=p_t[:], in0=e_t[:], scalar1=rd[:])
                pT = msgh.tile([T, T], BF16, tag="ms_pT", name="ms_pT")
                tp16(msps, p_t[:], T, T, pT[:], "msg_ps")
                ps_o = msps.tile([T, 64], F32, tag="msg_ps", name="msg_ps")
                nc.tensor.matmul(ps_o[:], pT[:], v_sl, start=True, stop=True)
                nc.any.tensor_copy(out=mo[:, h * 64:(h + 1) * 64], in_=ps_o[:])

            moT = msgp.tile([128, DC, T], BF16, name="moT")
            for kc in range(DC):
                tp16(msps, mo[:, kc * 128:(kc + 1) * 128], T, 128, moT[:, kc, :], "msg_ps")
            mwo_t = msgp.tile([128, DC, D], BF16, name="mwo_t")
            nc.sync.dma_start(out=mwo_t[:], in_=mwoh[:])
            msg_tok = persist.tile([T, D], F32, name="msg_tok")
            for oc in range(2):
                osl = slice(oc * 384, (oc + 1) * 384)
                ps = msps.tile([T, 384], F32, tag="msg_ps", name="msg_ps")
                for kc in range(DC):
                    nc.tensor.matmul(ps[:], moT[:, kc, :], mwo_t[:, kc, osl],
                                     start=(kc == 0), stop=False)
                nc.tensor.matmul(ps[:], ones_r[0:1, 0:T], mob_t[0:1, osl],
                                 start=False, stop=True)
                nc.vector.tensor_add(out=msg_tok[:, osl], in0=m0[:, osl], in1=ps[:])
            msps.release()
            msgh.release()
            msgp.release()

            # =========================================================
            # LN1 (regular tokens) -> ln1T8 [128, 6, 792] fp8
            # free layout: [0:788] regular tokens (c*197+l), [788:792] msg
            # =========================================================
            ln1T_pool = pool("ln1Tp", 1)
            cln1 = pool("cln1", 1)
            if "ln1" in ln_trivial:
                ln1g_t = ln1b_t = None
            else:
                ln1g_t, ln1b_t = ln_params(cln1, ln1g, ln1b)
            ps_ln1 = pool("ps_ln1", 3, "PSUM")

            ln1T8 = ln1T_pool.tile([128, DC, PNT], FP8, name="ln1T8")
            for c in range(CPC):
                nc.vector.memset(ln1T8[:, :, c * CB + L:(c + 1) * CB], 0.0)
            nc.vector.memset(ln1T8[:, :, MSG0 + CPC:PNT], 0.0)
            for c in range(CPC):
                for lc, (l0, nl) in enumerate(MCHUNKS):
                    lnb = lnscr.tile([128, D], BF16, tag="ln_tok", name="ln_tok")
                    layernorm(xc_tok[c][lc][0:nl, :], nl, ln1g_t, ln1b_t, lnb[0:nl, :])
                    for kc in range(DC):
                        tp16(ps_ln1, lnb[0:nl, kc * 128:(kc + 1) * 128], nl, 128,
                             ln1T8[:, kc, c * CB + l0: c * CB + l0 + nl], "ln_tp")

            # =========================================================
            # main qkv pass A (788 regular tokens; no msg dependency)
            # fp8 DoubleRow; qkvT keeps the interleaved (c*198+l) layout
            # =========================================================
            qkvT_pool = pool("qkvTp", 1, side="right")
            ps_qkv = pool("ps_qkv", 4, "PSUM")

            qkvT = [qkvT_pool.tile([128, NT], BF16, tag=f"qkvT_{oc}", name=f"qkvT_{oc}")
                    for oc in range(12)]
            OC_ORDER = [tq + 6 * j for tq in range(6) for j in range(2)]
            for oc in OC_ORDER:
                for hf in range(2):
                    tsl = slice(hf * PH, (hf + 1) * PH)
                    ps = ps_qkv.tile([128, PH], F32, tag="qkv_ps", name="qkv_ps")
                    for sp in range(0, DC, 2):
                        nc.tensor.matmul(ps[:], wq_t[oc][:, sp:sp + 2, :],
                                         ln1T8[:, sp:sp + 2, tsl],
                                         start=(sp == 0), stop=(sp == DC - 2),
                                         perf_mode=DR)
                    for ci in range(2):
                        c = hf * 2 + ci
                        nc.vector.tensor_scalar_add(
                            out=qkvT[oc][:, c * LC: c * LC + L],
                            in0=ps[:, ci * CB: ci * CB + L],
                            scalar1=qkvb_t[:, oc:oc + 1])

            # ---- token-major V (fp8 DR): vtok65[c][ck] [nk, 12, 65] ----
            vq_pool = pool("vqp", 1)
            ps_v = pool("ps_vt", 2, "PSUM")
            vtok = [[vq_pool.tile([128, H, 65], BF16, tag=f"vt_{c}_{ck}",
                                  name=f"vt_{c}_{ck}") for ck in range(2)]
                    for c in range(CPC)]
            for c in range(CPC):
                for ck in range(2):
                    nc.vector.memset(vtok[c][ck][:, :, 64:65], 1.0)
            qkvb_r = consts.tile([1, 3 * D], BF16, name="qkvb_r")
            nc.vector.tensor_copy(out=qkvb_r[0:1, :],
                                  in_=qkvb_t[:].rearrange("p c -> 1 (c p)"))

            def vtok_mm(tok0, ntk, dst_rows, c, msg_col=None):
                # token-major V for ntk tokens starting at ln1T8 col tok0
                for vh in range(2):
                    vf = slice(2 * D + vh * 384, 2 * D + (vh + 1) * 384)
                    ps = ps_v.tile([128, 384], F32, tag="vt_ps", name="vt_ps")
                    src_c = (ln1T8[:, :, tok0:tok0 + ntk] if msg_col is None
                             else ln1T8[:, :, msg_col:msg_col + 1])
                    for sp in range(0, DC, 2):
                        nc.tensor.matmul(ps[0:ntk, :],
                                         src_c[:, sp:sp + 2, :],
                                         wq_t[12 + vh * 3 + sp // 2][:, :, :].rearrange(
                                             "p kc o -> p (kc o)")[:, 0:0],
                                         start=False, stop=False)
                    nc.vector.memset(ps[0:1, 0:1], 0.0)
            # (placeholder removed below)

            # ---- LN1 of the 4 msg tokens + qkv pass B ----
            lnb4 = lnscr.tile([128, D], BF16, tag="ln_tok", name="ln_tok")
            layernorm(msg_tok[0:CPC, :], CPC, ln1g_t, ln1b_t, lnb4[0:CPC, :])
            for kc in range(DC):
                tp16(ps_ln1, lnb4[0:CPC, kc * 128:(kc + 1) * 128], CPC, 128,
                     ln1T8[:, kc, MSG0:MSG0 + CPC], "ln_tp")
            for oc in OC_ORDER:
                ps = ps_qkv.tile([128, CPC], F32, tag="qkv_ps", name="qkv_ps")
                for sp in range(0, DC, 2):
                    nc.tensor.matmul(ps[:], wq_t[oc][:, sp:sp + 2, :],
                                     ln1T8[:, sp:sp + 2, MSG0:MSG0 + CPC],
                                     start=(sp == 0), stop=(sp == DC - 2),
                                     perf_mode=DR)
                nc.scalar.activation(
                    out=qkvT[oc].rearrange("p (c l) -> p c l", c=CPC)[:, :, L],
                    in_=ps[:], func=AF.Identity, bias=qkvb_t[:, oc:oc + 1], scale=1.0)
            ps_qkv.release()
            ps_ln1.release()
            cln1.release()
            ln1T_pool.release()
            wq_pool.release()

            # =========================================================
            # attention per (h, c): merged key chunks, PSUM-accumulated AV,
            # per-head 1/den broadcast -> attT8 [128, 6, 788] fp8
            # =========================================================
            attT_pool = pool("attTp", 1)
            attT8 = attT_pool.tile([128, DC, PNT2], FP8, name="attT8")
            for c in range(CPC):
                nc.vector.memset(attT8[:, :, c * CB + L:(c + 1) * CB], 0.0)
            atp = pool("atp", 3)
            oap = pool("oap", 8)
            denp = pool("denp", 2)
            v65p = pool("v65p", 1)
            psA = pool("psA", 2, "PSUM")
            psB = pool("psB", 2, "PSUM")
            psO = pool("psO", 2, "PSUM")
            psC = pool("psC", 2, "PSUM")

            v65 = [v65p.tile([128, 65], BF16, tag=f"v65_{i}", name=f"v65_{i}")
                   for i in range(4)]
            for i in range(4):
                nc.vector.memset(v65[i][:, 64:65], 1.0)
            v65_idx = 0

            KCHUNKS = [(0, 128), (128, 70)]   # key chunks incl msg token
            for h in range(H):
                tq, of = h // 2, (h % 2) * 64
                den_h = denp.tile([1, PNT2], F32, tag="den_h", name="den_h")
                for c in range(CPC):
                    nc.vector.memset(den_h[0:1, c * CB + L:(c + 1) * CB], 1.0)
                oa_c = []
                for c in range(CPC):
                    csl0 = slice(c * LC, c * LC + L)   # queries exclude msg
                    qT = qkvT[tq][of:of + 64, csl0]
                    kT = qkvT[6 + tq][of:of + 64, c * LC: (c + 1) * LC]
                    vT = qkvT[12 + tq][of:of + 64, c * LC: (c + 1) * LC]
                    ps_oa = psO.tile([65, L], F32, tag="at_oa", name="at_oa")
                    for ck, (k0, nk) in enumerate(KCHUNKS):
                        ps_s = psA.tile([128, L], F32, tag="at_s", name="at_s")
                        nc.tensor.matmul(ps_s[0:nk, :], kT[:, k0:k0 + nk], qT,
                                         start=True, stop=True)
                        e_t = atp.tile([128, L], BF16, tag="at_e", name="at_e")
                        nc.scalar.activation(out=e_t[0:nk, :], in_=ps_s[0:nk, :],
                                             func=AF.Exp, scale=0.125)
                        ps_v = psB.tile([128, 64], BF16, tag="at_vps", name="at_vps")
                        nc.tensor.transpose(ps_v[0:nk, :], vT[:, k0:k0 + nk],
                                            id16_t[of:of + 64, of:of + 64])
                        vt = v65[v65_idx % 4]
                        v65_idx += 1
                        nc.any.tensor_copy(out=vt[0:nk, 0:64], in_=ps_v[0:nk, :])
                        nc.tensor.matmul(ps_oa[:], vt[0:nk, :], e_t[0:nk, :],
                                         start=(ck == 0), stop=(ck == 1))
                    oa = oap.tile([65, L], F32, tag="at_oac", name="at_oac")
                    nc.any.tensor_copy(out=oa[:], in_=ps_oa[:])
                    nc.vector.tensor_copy(out=den_h[0:1, c * CB: c * CB + L],
                                          in_=oa[64:65, :])
                    oa_c.append(oa)
                den_r = denp.tile([1, PNT2], BF16, tag="den_r", name="den_r")
                with nc.allow_low_precision(reason="softmax 1/den in bf16 (0.4% scale)"):
                    nc.vector.reciprocal(out=den_r[0:1, :], in_=den_h[0:1, :])
                bc_ps = []
                for hf in range(2):
                    tsl = slice(hf * PH, (hf + 1) * PH)
                    pb = psC.tile([64, PH], F32, tag="at_bc", name="at_bc")
                    nc.tensor.matmul(pb[:], ones_r[0:1, 0:64], den_r[0:1, tsl],
                                     start=True, stop=True)
                    bc_ps.append(pb)
                for c in range(CPC):
                    nc.vector.tensor_mul(
                        out=attT8[of:of + 64, tq, c * CB: c * CB + L],
                        in0=oa_c[c][0:64, :],
                        in1=bc_ps[c // 2][0:64, (c % 2) * CB:(c % 2) * CB + L])
            if debug_taps:
                nc.sync.dma_start(out=dbg["dbg_qkv"][:], in_=qkvT[0][:])
            psC.release()
            psO.release()
            psB.release()
            psA.release()
            v65p.release()
            denp.release()
            oap.release()
            atp.release()
            qkvT_pool.release()

            # =========================================================
            # attention out-proj (fp8 DoubleRow) + residual into xc_tok
            # =========================================================
            ps_pr = pool("ps_pr", 4, "PSUM")
            for c in range(CPC):
                for lc, (l0, nl) in enumerate(MCHUNKS):
                    tb = c * CB + l0
                    for oc in range(2):
                        osl = slice(oc * 384, (oc + 1) * 384)
                        ps = ps_pr.tile([128, 384], F32, tag="pr_ps", name="pr_ps")
                        for sp in range(0, DC, 2):
                            nc.tensor.matmul(ps[0:nl, :],
                                             attT8[:, sp:sp + 2, tb:tb + nl],
                                             wo_t[:, sp:sp + 2, osl],
                                             start=(sp == 0), stop=False,
                                             perf_mode=DR)
                        nc.tensor.matmul(ps[0:nl, :], ones_r[0:1, 0:nl], wob_t[0:1, osl],
                                         start=False, stop=True)
                        nc.vector.tensor_add(out=xc_tok[c][lc][0:nl, osl],
                                             in0=xc_tok[c][lc][0:nl, osl], in1=ps[0:nl, :])
            if debug_taps:
                nc.sync.dma_start(out=dbg["dbg_att"][:], in_=attT8[:, 0, :])
            ps_pr.release()
            attT_pool.release()
            wo_pool.release()

            # =========================================================
            # LN2 -> ln2T [kc][128, 788] bf16
            # =========================================================
            wtp = pool("wtp", 10)
            whp = pool("whp", 30)
            ln2T_pool = pool("ln2Tp", 1)
            cln2 = pool("cln2", 1)
            if "ln2" in ln_trivial:
                ln2g_t = ln2b_t = None
            else:
                ln2g_t, ln2b_t = ln_params(cln2, ln2g, ln2b)
            ps_ln2 = pool("ps_ln2", 6, "PSUM")

            ln2T = [ln2T_pool.tile([128, PNT2], BF16, tag=f"ln2T_{kc}", name=f"ln2T_{kc}")
                    for kc in range(DC)]
            for kc in range(DC):
                for c in range(CPC):
                    nc.vector.memset(ln2T[kc][:, c * CB + L:(c + 1) * CB], 0.0)
            for c in range(CPC):
                for lc, (l0, nl) in enumerate(MCHUNKS):
                    lnb = lnscr.tile([128, D], BF16, tag="ln_tok", name="ln_tok")
                    layernorm(xc_tok[c][lc][0:nl, :], nl, ln2g_t, ln2b_t, lnb[0:nl, :])
                    for kc in range(DC):
                        tp16(ps_ln2, lnb[0:nl, kc * 128:(kc + 1) * 128], nl, 128,
                             ln2T[kc][:, c * CB + l0: c * CB + l0 + nl], "ln_tp")
            ps_ln2.release()
            cln2.release()

            # =========================================================
            # routers + MoE
            # =========================================================
            rper = pool("rper", 1, side="right")
            oh_pool = pool("ohp", 1, side="right")
            r1w_t = rper.tile([128, DC, E1], BF16, name="r1w_t")
            nc.sync.dma_start(out=r1w_t[:], in_=r1wh[:])
            r2w_t = rper.tile([128, FC, 16], FP8, name="r2w_t")
            nc.vector.memset(r2w_t[:, :, E1:16], 0.0)
            nc.sync.dma_start(out=r2w_t[:, :, 0:E1], in_=r2wh[:])
            bh_t = rper.tile([E1, F], BF16, name="bh_t")
            nc.sync.dma_start(out=bh_t[:], in_=bhE[:])
            bt_t = rper.tile([E1, D], BF16, name="bt_t")
            nc.sync.dma_start(out=bt_t[:], in_=btE[:])

            def router_softmax(ps_r, ps, nl, tb, bias_row, dstT, skip_group=False):
                nc.tensor.matmul(ps[0:nl, :], ones_r[0:1, 0:nl], bias_row[0:1, :],
                                 start=False, stop=True, skip_group_check=skip_group)
                er = lnscr.tile([128, E1], BF16, tag="r_e", name="r_e")
                den = lnscr.tile([128, 1], F32, tag="r_den", name="r_den")
                nc.scalar.activation(out=er[0:nl, :], in_=ps[0:nl, :],
                                     func=AF.Exp, accum_out=den[0:nl, :])
                rdd = lnscr.tile([128, 1], F32, tag="r_rd", name="r_rd")
                nc.vector.reciprocal(out=rdd[0:nl, :], in_=den[0:nl, :])
                rn = lnscr.tile([128, E1], BF16, tag="r_n", name="r_n")
                nc.vector.tensor_scalar_mul(out=rn[0:nl, :], in0=er[0:nl, :],
                                            scalar1=rdd[0:nl, :])
                tp16(ps_r, rn[0:nl, :], nl, E1, dstT[0:E1, tb:tb + nl], "r_tp")

            # ---- router 1 + xeT ----
            ps_r1 = pool("ps_r1", 2, "PSUM")
            r1nT = rper.tile([E1, PNT2], BF16, name="r1nT")
            for c in range(CPC):
                nc.vector.memset(r1nT[0:E1, c * CB + L:(c + 1) * CB], 0.0)
            for c in range(CPC):
                for lc, (l0, nl) in enumerate(MCHUNKS):
                    tb = c * CB + l0
                    ps = ps_r1.tile([128, E1], F32, tag="r_ps", name="r_ps")
                    for kc in range(DC):
                        nc.tensor.matmul(ps[0:nl, :], ln2T[kc][:, tb:tb + nl],
                                         r1w_t[:, kc, :], start=(kc == 0), stop=False)
                    router_softmax(ps_r1, ps, nl, tb, r1b_t, r1nT)
            r1row = [rper.tile([1, PNT2], BF16, tag=f"r1row_{e}", name=f"r1row_{e}")
                     for e in range(E1)]
            for e in range(E1):
                nc.sync.dma_start(out=r1row[e][0:1, :], in_=r1nT[e:e + 1, :])

            xeT_pool = pool("xeTp", 1, side="right")
            xeT = [xeT_pool.tile([128, DC, PNT2], FP8, tag=f"xeT_{e}", name=f"xeT_{e}")
                   for e in range(E1)]
            for e in range(E1):
                for hf in range(2):
                    tsl = slice(hf * PH, (hf + 1) * PH)
                    ps_bc = ps_r1.tile([128, PH], F32, tag="bc_ps", name="bc_ps")
                    nc.tensor.matmul(ps_bc[:], ones_r[0:1, 0:128], r1row[e][0:1, tsl],
                                     start=True, stop=True)
                    bcb = lnscr.tile([128, PH], BF16, tag="bc_b", name="bc_b")
                    nc.scalar.copy(out=bcb[:], in_=ps_bc[:])
                    for kc in range(DC):
                        nc.vector.tensor_mul(out=xeT[e][:, kc, tsl],
                                             in0=ln2T[kc][:, tsl], in1=bcb[:])
            if debug_taps:
                nc.sync.dma_start(out=dbg["dbg_ln2"][:], in_=ln2T[0][:])
                nc.sync.dma_start(out=dbg["dbg_r1"][:], in_=r1nT[0:E1, :])
                nc.sync.dma_start(out=dbg["dbg_xe"][:], in_=xeT[0][:, 0, :])
            ln2T_pool.release()
            ps_r1.release()

            # ---- MoE head mms (fp8 DoubleRow) + qgelu -> oh8 ----
            ps_h = pool("ps_h", 4, "PSUM")
            oh8 = oh_pool.tile([128, FC, PNT2], FP8, name="oh8")
            for fc in range(FC):
                wh_tiles = []
                for e in range(E1):
                    wt_ = whp.tile([128, DC, 128], FP8, tag="wh_s", name="wh_s")
                    nc.sync.dma_start(out=wt_[:], in_=whh[e, fc])
                    wh_tiles.append(wt_)
                for hf in range(2):
                    tsl = slice(hf * PH, (hf + 1) * PH)
                    ps = ps_h.tile([128, PH], F32, tag="mh_ps", name="mh_ps")
                    first = True
                    for e in range(E1):
                        for sp in range(0, DC, 2):
                            nc.tensor.matmul(ps[:], wh_tiles[e][:, sp:sp + 2, :],
                                             xeT[e][:, sp:sp + 2, tsl],
                                             start=first, stop=False, perf_mode=DR)
                            first = False
                    nc.tensor.matmul(ps[:], bh_t[:, fc * 128:(fc + 1) * 128], r1nT[:, tsl],
                                     start=False, stop=True)
                    if gelu_compose:
                        sg = lnscr.tile([128, PH], BF16, tag="sg", name="sg")
                        nc.scalar.activation(out=sg[:], in_=ps[:],
                                             func=AF.Sigmoid, scale=1.702)
                        nc.vector.tensor_mul(out=oh8[:, fc, tsl], in0=ps[:], in1=sg[:])
                    else:
                        nc.scalar.activation(out=oh8[:, fc, tsl], in_=ps[:],
                                             func=AF.Gelu_apprx_sigmoid)
            if debug_taps:
                nc.sync.dma_start(out=dbg["dbg_oh"][:], in_=oh8[:, 0, :])
            ps_h.release()
            whp.release()
            xeT_pool.release()

            # ---- router 2 (fp8 DoubleRow) ----
            ps_r2 = pool("ps_r2", 2, "PSUM")
            r2nT = rper.tile([E1, PNT2], BF16, name="r2nT")
            for c in range(CPC):
                nc.vector.memset(r2nT[0:E1, c * CB + L:(c + 1) * CB], 0.0)
            for c in range(CPC):
                for lc, (l0, nl) in enumerate(MCHUNKS):
                    tb = c * CB + l0
                    ps = ps_r2.tile([128, E1], F32, tag="r_ps", name="r_ps")
                    for sp in range(0, FC, 2):
                        nc.tensor.matmul(ps[0:nl, :], oh8[:, sp:sp + 2, tb:tb + nl],
                                         r2w_t[:, sp:sp + 2, 0:E1],
                                         start=(sp == 0), stop=False, perf_mode=DR)
                    router_softmax(ps_r2, ps, nl, tb, r2b_t, r2nT)
            r2row = [rper.tile([1, PNT2], BF16, tag=f"r2row_{e}", name=f"r2row_{e}")
                     for e in range(E1)]
            for e in range(E1):
                nc.sync.dma_start(out=r2row[e][0:1, :], in_=r2nT[e:e + 1, :])
            # r2 routing weights broadcast across partitions, in SBUF (bf16-exact)
            bc2 = [rper.tile([128, PNT2], BF16, tag=f"bc2_{e}", name=f"bc2_{e}")
                   for e in range(E1)]
            for e in range(E1):
                for hf in range(2):
                    tsl = slice(hf * PH, (hf + 1) * PH)
                    ps_bc = ps_r2.tile([128, PH], F32, tag="bc_ps", name="bc_ps")
                    nc.tensor.matmul(ps_bc[:], ones_r[0:1, 0:128], r2row[e][0:1, tsl],
                                     start=True, stop=True)
                    nc.scalar.copy(out=bc2[e][:, tsl], in_=ps_bc[:])
            ps_r2.release()

            # ---- MoE tails (fp8 DoubleRow, output-scaled) + residual + store ----
            accp = pool("accp", 1, side="right")
            ps_t = pool("ps_t", 2, "PSUM")

            acc = [accp.tile([128, PNT2], F32, tag=f"acc_{fc2}", name=f"acc_{fc2}")
                   for fc2 in range(DC)]
            for fc2 in range(DC):
                ps_b = {}
                for hf in range(2):
                    tsl = slice(hf * PH, (hf + 1) * PH)
                    pb = ps_t.tile([128, PH], F32, tag="tl_bias", name="tl_bias")
                    nc.tensor.matmul(pb[:], bt_t[:, fc2 * 128:(fc2 + 1) * 128],
                                     r2nT[:, tsl], start=True, stop=True)
                    ps_b[hf] = pb
                for e in range(E1):
                    wtt = wtp.tile([128, FC, 128], FP8, tag="wt_s", name="wt_s")
                    nc.sync.dma_start(out=wtt[:], in_=wth[e, fc2])
                    for hf in range(2):
                        tsl = slice(hf * PH, (hf + 1) * PH)
                        ps_e = ps_t.tile([128, PH], F32, tag="tl_ps", name="tl_ps")
                        for sp in range(0, FC, 2):
                            nc.tensor.matmul(ps_e[:], wtt[:, sp:sp + 2, :],
                                             oh8[:, sp:sp + 2, tsl],
                                             start=(sp == 0), stop=(sp == FC - 2),
                                             perf_mode=DR)
                        tmp = lnscr.tile([128, PH], F32, tag="tl_tmp", name="tl_tmp")
                        if e == 0:
                            nc.vector.tensor_mul(out=acc[fc2][:, tsl], in0=ps_e[:],
                                                 in1=bc2[e][:, tsl])
                        else:
                            nc.vector.tensor_mul(out=tmp[:], in0=ps_e[:],
                                                 in1=bc2[e][:, tsl])
                            nc.gpsimd.tensor_add(out=acc[fc2][:, tsl],
                                                 in0=acc[fc2][:, tsl], in1=tmp[:])
                for hf in range(2):
                    tsl = slice(hf * PH, (hf + 1) * PH)
                    nc.vector.tensor_add(out=acc[fc2][:, tsl],
                                         in0=acc[fc2][:, tsl], in1=ps_b[hf][:])

            if debug_taps:
                nc.sync.dma_start(out=dbg["dbg_r2"][:], in_=r2nT[0:E1, :])
                nc.sync.dma_start(out=dbg["dbg_acc"][:], in_=acc[0][:])
            wtp.release()
            outp = pool("outp", 1)
            ps_o = pool("ps_o", 2, "PSUM")

            # final: out = x + moe; per-fc2 so output work overlaps later tails
            ot_tiles = {}
            for c in range(CPC):
                for lc in range(2):
                    ot_tiles[(c, lc)] = outp.tile([128, D], F32,
                                                  tag=f"out_{c}_{lc}", name=f"out_{c}_{lc}")
            for fc2 in range(DC):
                for c in range(CPC):
                    for lc, (l0, nl) in enumerate(MCHUNKS):
                        tb = c * CB + l0
                        ps_f = ps_o.tile([128, 128], F32, tag="out_tp", name="out_tp")
                        nc.tensor.transpose(ps_f[0:nl, :], acc[fc2][:, tb:tb + nl],
                                            id32_t[0:128, 0:128])
                        nc.vector.tensor_add(
                            out=ot_tiles[(c, lc)][0:nl, fc2 * 128:(fc2 + 1) * 128],
                            in0=xc_tok[c][lc][0:nl, fc2 * 128:(fc2 + 1) * 128],
                            in1=ps_f[0:nl, :])
            for c in range(CPC):
                for lc, (l0, nl) in enumerate(MCHUNKS):
                    nc.sync.dma_start(out=out[l0:l0 + nl, c, :],
                                      in_=ot_tiles[(c, lc)][0:nl, :])

            ps_o.release()
            ps_t.release()
            outp.release()
            accp.release()
            oh_pool.release()
            rper.release()
            lnscr.release()
            consts.release()
            persist.release()

        if loop_n:
            with tc.For_i(0, loop_n, 1):
                _body(0)
        else:
            for rep in range(reps):
                _body(rep)

    nc.finalize()
    return nc


def _prep_inputs(inputs):
    """Host-side: transpose/stack/cast weights, build per-core in_maps."""
    bf = ml_dtypes.bfloat16
    f8 = ml_dtypes.float8_e4m3
    f32 = np.float32

    def tb(a):
        return np.ascontiguousarray(np.asarray(a, f32).T).astype(bf)

    def t8(a):
        return np.ascontiguousarray(np.asarray(a, f32).T).astype(f8)

    x = np.asarray(inputs["x"], f32)              # (197, 32, 768)
    Wh = np.concatenate([np.asarray(inputs["cfc_w"], f32)[None],
                         np.asarray(inputs["eh_w"], f32)], 0)     # (5, 3072, 768)
    bh = np.concatenate([np.asarray(inputs["cfc_b"], f32)[None],
                         np.asarray(inputs["eh_b"], f32)], 0)     # (5, 3072)
    Wt = np.concatenate([np.asarray(inputs["cproj_w"], f32)[None],
                         np.asarray(inputs["et_w"], f32)], 0)     # (5, 768, 3072)
    btl = np.concatenate([np.asarray(inputs["cproj_b"], f32)[None],
                          np.asarray(inputs["et_b"], f32)], 0)    # (5, 768)

    def tile_kpo(aT, nblk):
        # aT [K, O] -> [O//nblk? ...] pre-tiled [nO, 128, K//128, nblk]
        Kd, Od = aT.shape
        return np.ascontiguousarray(
            aT.reshape(Kd // 128, 128, Od // nblk, nblk).transpose(2, 1, 0, 3))

    def tile_kO(aT):
        # aT [K, O] -> [128, K//128, O]
        Kd, Od = aT.shape
        return np.ascontiguousarray(aT.reshape(Kd // 128, 128, Od).transpose(1, 0, 2))

    wq_tiled = tile_kpo(t8(inputs["attn_wqkv"]), 128)          # (18,128,6,128)
    whT8 = np.ascontiguousarray(Wh.transpose(0, 2, 1)).astype(f8)   # (5,768,3072)
    wtT8 = np.ascontiguousarray(Wt.transpose(0, 2, 1)).astype(f8)   # (5,3072,768)
    whh = np.ascontiguousarray(
        whT8.reshape(E1, DC, 128, FC, 128).transpose(0, 3, 2, 1, 4))
    wth = np.ascontiguousarray(
        wtT8.reshape(E1, FC, 128, DC, 128).transpose(0, 3, 2, 1, 4))

    shared = {
        "id32": np.eye(128, dtype=f32),
        "id16": np.eye(128, dtype=f32).astype(bf),
        "mfcwh": tile_kO(tb(inputs["msg_fc_w"])),
        "mqkvh": tile_kO(tb(inputs["msg_attn_wqkv"])),
        "mwoh": tile_kO(tb(inputs["msg_attn_wo"])),
        "wqh": wq_tiled,
        "woh": tile_kO(t8(inputs["attn_wo"])),
        "whh": whh,
        "wth": wth,
        "r1wh": tile_kO(tb(inputs["r1_w"])),
        "r2wh": tile_kO(t8(inputs["r2_w"])),
        "bhE": bh.astype(bf),
        "btE": btl.astype(bf),
        "mfcb_r": np.asarray(inputs["msg_fc_b"], f32)[None].astype(bf),
        "mqkvb_r": np.asarray(inputs["msg_attn_bqkv"], f32)[None].astype(bf),
        "mob_r": np.asarray(inputs["msg_attn_bo"], f32)[None].astype(bf),
        "wob_r": np.asarray(inputs["attn_bo"], f32)[None].astype(bf),
        "r1b_r": np.asarray(inputs["r1_b"], f32)[None].astype(bf),
        "r2b_r": np.asarray(inputs["r2_b"], f32)[None].astype(bf),
        "qkvb": np.asarray(inputs["attn_bqkv"], f32),
        "mlng": np.asarray(inputs["msg_ln_g"], f32),
        "mlnb": np.asarray(inputs["msg_ln_b"], f32),
        "ln1g": np.asarray(inputs["ln1_g"], f32),
        "ln1b": np.asarray(inputs["ln1_b"], f32),
        "ln2g": np.asarray(inputs["ln2_g"], f32),
        "ln2b": np.asarray(inputs["ln2_b"], f32),
    }

    in_maps = []
    for core in range(NCORES):
        c0 = core * CPC
        b = c0 // T
        off = c0 - b * T
        order = list(range(off, off + CPC)) + \
            [i for i in range(T) if not (off <= i < off + CPC)]
        x0cv = x[0, b * T: (b + 1) * T, :][order]
        m = dict(shared)
        m["xs"] = np.ascontiguousarray(x[:, c0:c0 + CPC, :])
        m["x0c"] = np.ascontiguousarray(x0cv)
        in_maps.append(m)
    return in_maps


def _ln_trivial_flags(inputs):
    flags = []
    checks = {"mln": ("msg_ln_g", "msg_ln_b"), "ln1": ("ln1_g", "ln1_b"),
              "ln2": ("ln2_g", "ln2_b")}
    for nm, (g, b) in checks.items():
        if np.allclose(np.asarray(inputs[g]), 1.0) and \
                np.allclose(np.asarray(inputs[b]), 0.0):
            flags.append(nm)
    return tuple(sorted(flags))


def kernel(**inputs):
    flags = _ln_trivial_flags(inputs)
    key = ("nc", flags)
    if key not in _CACHE:
        _CACHE[key] = build(ln_trivial=flags)
        _CACHE["nc"] = _CACHE[key]
    nc = _CACHE[key]
    expected_in = set()
    for alloc in nc.m.functions[0].allocations:
        if isinstance(alloc, mybir.MemoryLocationSet) and alloc.kind == "ExternalInput":
            expected_in.add(alloc.memorylocations[0].name)
    in_maps = [{k: v for k, v in m.items() if k in expected_in}
               for m in _prep_inputs(inputs)]
    res = run_bass_kernel_spmd(nc, in_maps, core_ids=list(range(NCORES)))
    out = np.concatenate([res.results[i]["out"] for i in range(NCORES)], axis=1)
    return out.astype(np.float32)
